# revision 1
# baseline (speedup 1.0000x reference)
"""BiLSTM-CRF loss kernel for Trainium2 (8 NeuronCores, data-parallel over batch).

Self-contained: hardcodes shapes B=128, T=512, V=50000, NT=24, E=128, H=256.
Each core processes 16 examples end-to-end (embedding gather, BiLSTM,
emissions, CRF forward logZ, gold path score); host only reorders inputs into
DMA-friendly layouts, builds one-hot index selectors, and averages the 128
per-example (logZ - gold) values.
"""

import sys

for _p in ("/opt/trn_rl_repo",):
    if _p not in sys.path:
        sys.path.insert(0, _p)

import numpy as np
import ml_dtypes

import concourse.bass as bass
import concourse.bacc as bacc
import concourse.tile as tile
from concourse import mybir
from concourse.bass import IndirectOffsetOnAxis
from concourse.masks import make_identity

F32 = mybir.dt.float32
BF16 = mybir.dt.bfloat16
I32 = mybir.dt.int32
U8 = mybir.dt.uint8
AX = mybir.AxisListType
OP = mybir.AluOpType
ACTF = mybir.ActivationFunctionType


def full_cfg():
    return dict(T=512, Bl=16, V=50000, NT=24, E=128, Hd=128, Tc=32, UT=256,
                EC=512, TG=32)


def shift_steps(cfg):
    # steps at which the CRF running score is re-based (every 4th step keeps
    # |log q| < ~60, safely inside f32 exp range)
    return [t for t in range(4, cfg["T"], 4)]


def build_body(tc, outs, ins, cfg):
    """Emit the whole per-core program inside an open TileContext.

    outs/ins: dicts name -> bass.AP (DRAM).
    """
    nc = tc.nc
    T, Bl, NT, Hd = cfg["T"], cfg["Bl"], cfg["NT"], cfg["Hd"]
    Tc, UT, EC, TG = cfg["Tc"], cfg["UT"], cfg["EC"], cfg["TG"]
    R = T * Bl                  # total (t, b) rows
    M = R // 128                # 128-row tiles
    NCH = T // Tc               # lstm chunks
    RTC = Tc * Bl // 128        # row-tiles per chunk
    G4 = 4 * Hd
    shifts = shift_steps(cfg)
    shift_of = {t: i for i, t in enumerate(shifts)}

    import contextlib
    ctx = contextlib.ExitStack()
    with ctx:
        const = ctx.enter_context(tc.tile_pool(name="const", bufs=1))
        big = ctx.enter_context(tc.tile_pool(name="big", bufs=1))
        work = ctx.enter_context(tc.tile_pool(name="work", bufs=3))

        # ---------------- constants ----------------
        ident = const.tile([128, 128], F32)
        make_identity(nc, ident[:])

        idx_sb = const.tile([128, M], I32)
        nc.sync.dma_start(out=idx_sb[:], in_=ins["idx"][:])

        wih_sb = const.tile([128, 2, G4], BF16)
        nc.sync.dma_start(out=wih_sb[:], in_=ins["wih"][:])
        whh_sb = const.tile([128, 2, G4], BF16)
        nc.sync.dma_start(out=whh_sb[:], in_=ins["whh"][:])
        wout_sb = const.tile([128, 2, NT], BF16)
        nc.sync.dma_start(out=wout_sb[:], in_=ins["wout"][:])

        biasin = const.tile([128, 2, 2, 4], F32)
        nc.sync.dma_start(out=biasin[:], in_=ins["biasin"][:])
        bias_sb = const.tile([128, 2, 4], F32)
        nc.vector.tensor_add(bias_sb[:], biasin[:, :, 0, :], biasin[:, :, 1, :])

        trans_sb = const.tile([NT, NT], F32)
        nc.sync.dma_start(out=trans_sb[:], in_=ins["trans"][:])
        E_sb = const.tile([NT, NT], F32)
        nc.scalar.activation(E_sb[:], trans_sb[:], ACTF.Exp)

        bout_sb = const.tile([NT, 1], F32)
        nc.sync.dma_start(out=bout_sb[:], in_=ins["bout"][:])
        startv = const.tile([NT, 1], F32)
        nc.sync.dma_start(out=startv[:], in_=ins["startv"][:])
        endv = const.tile([NT, 1], F32)
        nc.sync.dma_start(out=endv[:], in_=ins["endv"][:])
        selstart = const.tile([NT, Bl], F32)
        nc.sync.dma_start(out=selstart[:], in_=ins["selstart"][:])
        selend = const.tile([NT, Bl], F32)
        nc.sync.dma_start(out=selend[:], in_=ins["selend"][:])

        nsh = len(shifts)
        maskA = const.tile([1, nsh * Bl], F32)
        nc.sync.dma_start(out=maskA[:], in_=ins["maskA"][:])
        EV = cfg["EV"]
        snap_sb = const.tile([NT, max(len(EV), 1)], U8)
        nc.sync.dma_start(out=snap_sb[:], in_=ins["snapmask"][:])

        ones24 = const.tile([NT, 1], F32)
        nc.vector.memset(ones24[:], 1.0)
        ones1 = const.tile([1, NT], F32)
        nc.vector.memset(ones1[:], 1.0)

        # ---------------- phase A: gather + transpose embeddings ----------------
        xeT = big.tile([128, R], BF16)
        with tc.tile_pool(name="psA", bufs=1, space="PSUM") as psA, \
             tc.tile_pool(name="psG", bufs=3, space="PSUM") as psG, \
             tc.tile_pool(name="gath", bufs=3) as gath:
            gather_order = []
            for i in range((M + 1) // 2):
                gather_order.append(M - 1 - i)
                if i != M - 1 - i:
                    gather_order.append(i)
            for m in gather_order:
                xe_raw = gath.tile([128, 128], F32, tag="xe_raw")
                nc.gpsimd.indirect_dma_start(
                    out=xe_raw[:], out_offset=None,
                    in_=ins["emb"][:],
                    in_offset=IndirectOffsetOnAxis(ap=idx_sb[:, m:m + 1], axis=0),
                )
                xe_ps = psA.tile([128, 128], F32, tag="xe_ps")
                nc.tensor.transpose(xe_ps[:], xe_raw[:], ident[:])
                nc.vector.tensor_copy(xeT[:, m * 128:(m + 1) * 128], xe_ps[:])

            # ---------------- phase B: BiLSTM ----------------
            h_f = big.tile([128, R], BF16)
            h_b = big.tile([128, R], BF16)
            zero_h = const.tile([128, 2 * Bl], BF16)
            nc.vector.memset(zero_h[:], 0.0)
            c_f = big.tile([128, Bl], F32)
            nc.vector.memset(c_f[:], 0.0)
            c_b = big.tile([128, Bl], F32)
            nc.vector.memset(c_b[:], 0.0)

            # bias as a K=4 matmul: biasT [4, dir, 128] (transposed bias) and
            # a one-hot selector so one accumulating matmul adds bias[j,k] to
            # every (k, b) column of the gate PSUM tile.
            biasT = const.tile([4, 2, 128], BF16)
            sel4 = const.tile([4, 4 * Bl], BF16)
            nc.sync.dma_start(out=sel4[:], in_=ins["sel4"][:])
            for d in range(2):
                bt_ps = psA.tile([4, 128], F32, tag="bt_ps")
                nc.tensor.transpose(bt_ps[:], bias_sb[:, d, :], ident[:])
                nc.vector.tensor_copy(biasT[:, d, :], bt_ps[:])

            with tc.tile_pool(name="lwork", bufs=3) as lwork:
                # zero_b: bwd's initial h, made to *depend on* fwd's first
                # sigmoid so the bwd chain starts half a round later and the
                # two chains stay phase-offset (latencies are identical, so
                # the initial offset persists).
                zero_b = const.tile([128, Bl], BF16)
                pend_bwd = None
                if True:
                    def lstm_h1(d, t):
                        # first half: all of g accumulated in PSUM by the PE
                        # (recurrence + input projection + bias), then sigmoid
                        # straight off PSUM.
                        h_st = h_f if d == 0 else h_b
                        rhs = (h_st[:, (t - 1) * Bl:t * Bl] if d == 0 and t > 0
                               else h_st[:, (t + 1) * Bl:(t + 2) * Bl]
                               if d == 1 and t < T - 1 else
                               (zero_h[:, 0:Bl] if d == 0 else zero_b[:]))
                        gps = psG.tile([128, 4, Bl], F32, tag=f"gps{d}")
                        for k in range(4):
                            nc.tensor.matmul(
                                gps[:, k, :],
                                lhsT=whh_sb[:, d, k * 128:(k + 1) * 128],
                                rhs=rhs, start=(k == 0), stop=False)
                        for k in range(4):
                            nc.tensor.matmul(
                                gps[:, k, :],
                                lhsT=wih_sb[:, d, k * 128:(k + 1) * 128],
                                rhs=xeT[:, t * Bl:(t + 1) * Bl],
                                start=False, stop=False)
                        nc.tensor.matmul(
                            gps[:].rearrange("p k b -> p (k b)"),
                            lhsT=biasT[:, d, :], rhs=sel4[:],
                            start=False, stop=True)
                        # all-sigmoid cell: weights pre-scaled on host so
                        # tanh(x) = 2*sig(2x)-1 and h is stored as h/2.
                        sg = lwork.tile([128, 4, Bl], F32, tag=f"sg{d}")
                        nc.scalar.activation(sg[:], gps[:], ACTF.Sigmoid)
                        return sg

                    def lstm_h2(d, t, sg, c_st):
                        # second half: cell update + h output
                        h_st = h_f if d == 0 else h_b
                        m1 = lwork.tile([128, Bl], F32, tag=f"m1{d}")
                        nc.vector.scalar_tensor_tensor(
                            m1[:], sg[:, 3, :], 0.5, sg[:, 0, :],
                            op0=OP.subtract, op1=OP.mult)
                        m2 = lwork.tile([128, Bl], F32, tag=f"m2{d}")
                        nc.gpsimd.tensor_mul(m2[:], sg[:, 1, :], c_st)
                        nc.vector.scalar_tensor_tensor(
                            c_st, m1[:], 2.0, m2[:],
                            op0=OP.mult, op1=OP.add)
                        tcn = lwork.tile([128, Bl], F32, tag=f"tcn{d}")
                        nc.scalar.activation(tcn[:], c_st, ACTF.Sigmoid,
                                             scale=2.0)
                        nc.vector.scalar_tensor_tensor(
                            h_st[:, t * Bl:(t + 1) * Bl],
                            tcn[:], 0.5, sg[:, 2, :],
                            op0=OP.subtract, op1=OP.mult)

                    # software-pipelined emission: bwd runs half a step behind
                    # fwd so the two chains' engine visits interleave.
                    for s in range(T):
                        sgf = lstm_h1(0, s)
                        if s == 0:
                            nc.vector.tensor_scalar_mul(zero_b[:],
                                                        sgf[:, 0, :], 0.0)
                        if pend_bwd is not None:
                            lstm_h2(1, pend_bwd[0], pend_bwd[1], c_b[:])
                        sgb = lstm_h1(1, T - 1 - s)
                        lstm_h2(0, s, sgf, c_f[:])
                        pend_bwd = (T - 1 - s, sgb)
                lstm_h2(1, pend_bwd[0], pend_bwd[1], c_b[:])

        # ---------------- phase C: emissions ----------------
        emT = big.tile([NT, R], F32)
        EM = big.tile([NT, R], F32)
        with tc.tile_pool(name="psB", bufs=2, space="PSUM") as psB:
            n_ec = R // EC
            ec_t = EC // Bl
            ec_order = sorted(range(n_ec),
                              key=lambda c: max((c + 1) * ec_t,
                                                T - 1 - c * ec_t))
            for ec in ec_order:
                ps = psB.tile([NT, EC], F32, tag="em_ps")
                nc.tensor.matmul(ps[:], lhsT=wout_sb[:, 0, :],
                                 rhs=h_f[:, ec * EC:(ec + 1) * EC],
                                 start=True, stop=False)
                nc.tensor.matmul(ps[:], lhsT=wout_sb[:, 1, :],
                                 rhs=h_b[:, ec * EC:(ec + 1) * EC],
                                 start=False, stop=True)
                nc.vector.tensor_scalar_add(emT[:, ec * EC:(ec + 1) * EC],
                                            ps[:], bout_sb[:])
                nc.scalar.activation(EM[:, ec * EC:(ec + 1) * EC],
                                     emT[:, ec * EC:(ec + 1) * EC], ACTF.Exp)

        # ---------------- phase D: CRF forward + gold ----------------
        # exp-domain CRF: q_t = exp(score'_t); per step q <- (E^T q) * exp(em_t)
        # with periodic rebase q <- q / q[0] (offsets accumulated via logs at
        # the end). No activation-table switches inside the loop. EM is
        # exponentiated per emission chunk so the CRF overlaps emissions.
        with tc.tile_pool(name="psC", bufs=2, space="PSUM") as psC, \
             tc.tile_pool(name="psD", bufs=1, space="PSUM") as psD, \
             tc.tile_pool(name="cwork", bufs=3) as cwork, \
             tc.tile_pool(name="gwork", bufs=2) as gwork:
            estart = cwork.tile([NT, 1], F32, tag="estart")
            nc.scalar.activation(estart[:], startv[:], ACTF.Exp)
            NHALF = 1
            Hb = Bl // NHALF
            q_half = []
            q0_hist = big.tile([1, max(nsh, 1) * Bl], F32)
            for hh in range(NHALF):
                qh = big.tile([NT, Hb], F32)
                nc.vector.tensor_scalar_mul(
                    qh[:], EM[:, hh * Hb:(hh + 1) * Hb], estart[:])
                q_half.append(qh)

            def crf_step(hh, t):
                q = q_half[hh]
                o = hh * Hb
                if t in shift_of:
                    si = shift_of[t]
                    nc.vector.tensor_copy(
                        q0_hist[0:1, si * Bl + o:si * Bl + o + Hb], q[0:1, :])
                    rc = cwork.tile([1, Hb], F32, tag=f"rc{hh}")
                    nc.vector.reciprocal(rc[:], q[0:1, :])
                    ob = psD.tile([NT, Hb], F32, tag=f"ob{hh}")
                    nc.tensor.matmul(ob[:], lhsT=ones1[:], rhs=rc[:],
                                     start=True, stop=True)
                    qs = cwork.tile([NT, Hb], F32, tag=f"qs{hh}")
                    nc.vector.tensor_mul(qs[:], q[:], ob[:])
                    rhs_mm = qs
                else:
                    rhs_mm = q
                Pp = psC.tile([NT, Hb], F32, tag=f"Pp{hh}")
                nc.tensor.matmul(Pp[:], lhsT=E_sb[:], rhs=rhs_mm[:],
                                 start=True, stop=True)
                em_sl = EM[:, t * Bl + o:t * Bl + o + Hb]
                nc.vector.tensor_mul(q[:], Pp[:], em_sl)

            # per-example final-score snapshots: event e freezes column b at
            # step t; the per-core snapmask column enables only this core's
            # own freeze events (the event list is the union over cores).
            qfinal = big.tile([NT, Bl], F32)
            ev_at = {}
            for e, (t_, b_) in enumerate(EV):
                ev_at.setdefault(t_, []).append((e, b_))
            for t in range(1, T):
                for hh in range(NHALF):
                    crf_step(hh, t)
                for e, b_ in ev_at.get(t, []):
                    nc.vector.copy_predicated(
                        qfinal[:, b_:b_ + 1], snap_sb[:, e:e + 1],
                        q_half[b_ // Hb][:, b_ % Hb:b_ % Hb + 1])

            # logZ' = LSE_i(ln q_i + end_i) per example
            scoreT = cwork.tile([NT, Bl], F32, tag="scoreT")
            nc.scalar.activation(scoreT[:], qfinal[:], ACTF.Ln)
            sce = cwork.tile([NT, Bl], F32, tag="sce")
            nc.vector.tensor_scalar_add(sce[:], scoreT[:], endv[:])
            tp = psD.tile([Bl, NT], F32, tag="tiny")
            nc.tensor.transpose(tp[:], sce[:], ident[0:NT, 0:NT])
            sc2 = cwork.tile([Bl, NT], F32, tag="sc2")
            nc.vector.tensor_copy(sc2[:], tp[:])
            nmx = cwork.tile([Bl, 1], F32, tag="nmx")
            nc.vector.tensor_reduce(nmx[:], sc2[:], axis=AX.X, op=OP.max,
                                    negate=True)
            pe2 = cwork.tile([Bl, NT], F32, tag="pe2")
            se = cwork.tile([Bl, 1], F32, tag="se")
            nc.scalar.activation(pe2[:], sc2[:], ACTF.Exp, bias=nmx[:],
                                 accum_out=se[:])
            lse = cwork.tile([Bl, 1], F32, tag="lse")
            nc.scalar.activation(lse[:], se[:], ACTF.Ln)
            logZp = cwork.tile([Bl, 1], F32, tag="logZp")
            nc.vector.tensor_tensor(logZp[:], lse[:], nmx[:], op=OP.subtract)

            # A_b = sum over shift steps of ln(q0) * mask
            lnq0 = big.tile([1, max(nsh, 1) * Bl], F32)
            nc.scalar.activation(lnq0[:], q0_hist[:], ACTF.Ln)
            ohm = big.tile([1, max(nsh, 1) * Bl], F32)
            nc.vector.tensor_mul(ohm[:], lnq0[:], maskA[:])
            A_t = cwork.tile([1, Bl], F32, tag="A_t")
            ohm_v = ohm[:].rearrange("p (s b) -> p b s", b=Bl)
            nc.vector.tensor_reduce(A_t[:], ohm_v, axis=AX.X, op=OP.add)

            # ---- gold path score ----
            accE = gwork.tile([NT, Bl], F32, tag="accE")
            nc.vector.memset(accE[:], 0.0)
            accT = gwork.tile([NT, Bl], F32, tag="accT")
            nc.vector.memset(accT[:], 0.0)

            # emission term: sum_t w1hot[:, t, b] * emT[:, t, b]
            n_em_chunks = (T + TG - 1) // TG
            for gc in range(n_em_chunks):
                t0 = gc * TG
                nt_ = min(TG, T - t0)
                w1 = gwork.tile([NT, TG * Bl], F32, tag="w1")
                nc.sync.dma_start(
                    out=w1[:, 0:nt_ * Bl],
                    in_=ins["w1hot"][:, t0 * Bl:(t0 + nt_) * Bl])
                mm1 = gwork.tile([NT, TG * Bl], F32, tag="mm1")
                nc.vector.tensor_mul(mm1[:, 0:nt_ * Bl], w1[:, 0:nt_ * Bl],
                                     emT[:, t0 * Bl:(t0 + nt_) * Bl])
                mv = mm1[:, 0:nt_ * Bl].rearrange("p (t b) -> p b t", b=Bl)
                red = gwork.tile([NT, Bl], F32, tag="red")
                nc.vector.tensor_reduce(red[:], mv, axis=AX.X, op=OP.add)
                nc.vector.tensor_add(accE[:], accE[:], red[:])

            # transition term: sum_t sel2m[:,t,b] * (trans^T @ sel1)[:,t,b]
            n_tr_chunks = (T - 1 + TG - 1) // TG
            for gc in range(n_tr_chunks):
                t0 = gc * TG
                nt_ = min(TG, T - 1 - t0)
                s1 = gwork.tile([NT, TG * Bl], F32, tag="s1")
                nc.sync.dma_start(
                    out=s1[:, 0:nt_ * Bl],
                    in_=ins["sel1"][:, t0 * Bl:(t0 + nt_) * Bl])
                s2 = gwork.tile([NT, TG * Bl], F32, tag="s2")
                nc.sync.dma_start(
                    out=s2[:, 0:nt_ * Bl],
                    in_=ins["sel2m"][:, t0 * Bl:(t0 + nt_) * Bl])
                trp = psD.tile([NT, TG * Bl], F32, tag="trp")
                nc.tensor.matmul(trp[:, 0:nt_ * Bl], lhsT=trans_sb[:],
                                 rhs=s1[:, 0:nt_ * Bl], start=True, stop=True)
                mm2 = gwork.tile([NT, TG * Bl], F32, tag="mm2")
                nc.vector.tensor_mul(mm2[:, 0:nt_ * Bl], s2[:, 0:nt_ * Bl],
                                     trp[:, 0:nt_ * Bl])
                mv2 = mm2[:, 0:nt_ * Bl].rearrange("p (t b) -> p b t", b=Bl)
                red2 = gwork.tile([NT, Bl], F32, tag="red2")
                nc.vector.tensor_reduce(red2[:], mv2, axis=AX.X, op=OP.add)
                nc.vector.tensor_add(accT[:], accT[:], red2[:])

            nc.vector.tensor_add(accE[:], accE[:], accT[:])
            gsum = psD.tile([1, Bl], F32, tag="tiny")
            nc.tensor.matmul(gsum[:], lhsT=ones24[:], rhs=accE[:],
                             start=True, stop=False)
            nc.tensor.matmul(gsum[:], lhsT=startv[:], rhs=selstart[:],
                             start=False, stop=False)
            nc.tensor.matmul(gsum[:], lhsT=endv[:], rhs=selend[:],
                             start=False, stop=True)

            # r1 = gold - A   (loss = logZ' - r1)
            r1 = cwork.tile([1, Bl], F32, tag="r1")
            nc.vector.tensor_tensor(r1[:], gsum[:], A_t[:], op=OP.subtract)
            r1p = psD.tile([Bl, 1], F32, tag="tiny")
            nc.tensor.transpose(r1p[:], r1[:], ident[0:1, 0:1])
            loss = cwork.tile([Bl, 1], F32, tag="loss")
            nc.vector.tensor_tensor(loss[:], logZp[:], r1p[:], op=OP.subtract)
            nc.sync.dma_start(out=outs["loss"][:].unsqueeze(1), in_=loss[:])


# ======================= host-side preparation =======================

def make_core_inputs(cfg, x, tags, mask, emb, Wih_f, Whh_f, bih_f, bhh_f,
                     Wih_b, Whh_b, bih_b, bhh_b, W_out, b_out,
                     transitions, start_trans, end_trans):
    """Build the per-core input map (numpy). x/tags/mask are the LOCAL slices
    [Bl, T]."""
    T, Bl, NT, Hd, UT = cfg["T"], cfg["Bl"], cfg["NT"], cfg["Hd"], cfg["UT"]
    R = T * Bl
    M = R // 128
    perm = [0, 1, 3, 2]  # torch gate order (i,f,g,o) -> our (i,f,o,g)

    # Gate scale factors fold the all-sigmoid cell rewrite into the weights:
    # tanh(x)=2*sig(2x)-1 needs gate-g pre-activations doubled; h is stored
    # as h/2 so everything consuming h (Whh, W_out) is doubled.
    WIH_S = np.array([1.0, 1.0, 1.0, 2.0], np.float32)   # (i,f,o,g)
    WHH_S = np.array([2.0, 2.0, 2.0, 4.0], np.float32)
    BIA_S = WIH_S

    def reorder_rows(w, scales):  # [4Hd, ...] gate blocks
        blocks = [w[k * Hd:(k + 1) * Hd] * s for k, s in zip(perm, scales)]
        return np.concatenate(blocks, axis=0)

    def pack_w(wf, wb, scales):  # [4Hd, 128] each -> [128, 2, 4Hd] bf16
        out = np.empty((128, 2, 4 * Hd), dtype=ml_dtypes.bfloat16)
        out[:, 0, :] = reorder_rows(np.asarray(wf, np.float32), scales).T
        out[:, 1, :] = reorder_rows(np.asarray(wb, np.float32), scales).T
        return out

    def pack_bias(b):  # [4Hd] -> [128, 4]
        return reorder_rows(np.asarray(b, np.float32), BIA_S).reshape(4, Hd).T

    biasin = np.empty((128, 2, 2, 4), np.float32)
    biasin[:, 0, 0, :] = pack_bias(bih_f)
    biasin[:, 0, 1, :] = pack_bias(bhh_f)
    biasin[:, 1, 0, :] = pack_bias(bih_b)
    biasin[:, 1, 1, :] = pack_bias(bhh_b)

    W_out = np.asarray(W_out, np.float32) * 2.0   # h stored as h/2
    wout = np.empty((128, 2, NT), dtype=ml_dtypes.bfloat16)
    wout[:, 0, :] = W_out[:, :Hd].T
    wout[:, 1, :] = W_out[:, Hd:].T

    x = np.asarray(x)
    tags = np.asarray(tags)
    maskf = np.asarray(mask).astype(np.float32)

    # t-major row index: row r = t*Bl + b  ->  x[b, t]
    x_tm = x.T.reshape(-1).astype(np.int32)          # [R]
    idx = x_tm.reshape(M, 128).T.copy()              # [128, M]

    eye = np.eye(NT, dtype=np.float32)
    # w1hot[j, t*Bl+b] = onehot(tags[b,t])[j] * (1 if t==0 else mask[b,t])
    w = maskf.copy()
    w[:, 0] = 1.0
    w1 = eye[tags]                                   # [Bl, T, NT]
    w1 = (w1 * w[:, :, None]).transpose(2, 1, 0)     # [NT, T, Bl]
    w1hot = np.ascontiguousarray(w1.reshape(NT, R), np.float32)

    sel1 = eye[tags[:, :-1]].transpose(2, 1, 0).reshape(NT, (T - 1) * Bl)
    sel1 = np.ascontiguousarray(sel1, np.float32)
    sel2 = eye[tags[:, 1:]] * maskf[:, 1:, None]
    sel2m = np.ascontiguousarray(
        sel2.transpose(2, 1, 0).reshape(NT, (T - 1) * Bl), np.float32)

    selstart = np.ascontiguousarray(eye[tags[:, 0]].T, np.float32)  # [NT, Bl]
    last_idx = np.asarray(mask).sum(axis=1).astype(np.int64) - 1
    last_tags = tags[np.arange(Bl), last_idx]
    selend = np.ascontiguousarray(eye[last_tags].T, np.float32)

    shifts = shift_steps(cfg)
    maskA = maskf[:, shifts].T.reshape(1, -1).astype(np.float32)  # [1,nsh*Bl]
    EV = cfg["EV"]
    fz = np.asarray(mask).sum(axis=1).astype(np.int64) - 1   # freeze step per b
    snapmask = np.zeros((NT, max(len(EV), 1)), np.uint8)
    for e, (t_, b_) in enumerate(EV):
        if fz[b_] == t_:
            snapmask[:, e] = 1

    return {
        "emb": np.ascontiguousarray(emb, np.float32),
        "idx": idx,
        "wih": pack_w(Wih_f, Wih_b, WIH_S),
        "whh": pack_w(Whh_f, Whh_b, WHH_S),
        "wout": wout,
        "biasin": biasin,
        "bout": np.asarray(b_out, np.float32).reshape(NT, 1),
        "trans": np.ascontiguousarray(transitions, np.float32),
        "startv": np.asarray(start_trans, np.float32).reshape(NT, 1),
        "endv": np.asarray(end_trans, np.float32).reshape(NT, 1),
        "selstart": selstart,
        "selend": selend,
        "maskA": maskA,
        "snapmask": snapmask,
        "w1hot": w1hot,
        "sel1": sel1,
        "sel4": np.kron(np.eye(4, dtype=np.float32),
                        np.ones((1, Bl), np.float32)).astype(ml_dtypes.bfloat16),
        "sel2m": sel2m,
    }


def input_specs(cfg):
    T, Bl, NT, Hd, UT, V = (cfg["T"], cfg["Bl"], cfg["NT"], cfg["Hd"],
                            cfg["UT"], cfg["V"])
    R = T * Bl
    M = R // 128
    nsh = len(shift_steps(cfg))
    return {
        "emb": ([V, 128], F32),
        "idx": ([128, M], I32),
        "wih": ([128, 2, 4 * Hd], BF16),
        "whh": ([128, 2, 4 * Hd], BF16),
        "wout": ([128, 2, NT], BF16),
        "biasin": ([128, 2, 2, 4], F32),
        "bout": ([NT, 1], F32),
        "trans": ([NT, NT], F32),
        "startv": ([NT, 1], F32),
        "endv": ([NT, 1], F32),
        "selstart": ([NT, Bl], F32),
        "selend": ([NT, Bl], F32),
        "maskA": ([1, nsh * Bl], F32),
        "snapmask": ([NT, max(len(cfg["EV"]), 1)], U8),
        "w1hot": ([NT, R], F32),
        "sel1": ([NT, (T - 1) * Bl], F32),
        "sel4": ([4, 4 * cfg["Bl"]], BF16),
        "sel2m": ([NT, (T - 1) * Bl], F32),
    }


_BUILT = {}


def build_program(cfg, num_devices=8):
    key = tuple(sorted(cfg.items()))
    if key in _BUILT:
        return _BUILT[key]
    nc = bacc.Bacc("TRN2", target_bir_lowering=False, debug=False,
                   num_devices=num_devices)
    ins = {}
    for name, (shape, dt_) in input_specs(cfg).items():
        ins[name] = nc.dram_tensor(name, shape, dt_, kind="ExternalInput").ap()
    outs = {"loss": nc.dram_tensor("loss", [cfg["Bl"]], F32,
                                   kind="ExternalOutput").ap()}
    with tile.TileContext(nc) as tc:
        build_body(tc, outs, ins, cfg)
    nc.compile()
    _BUILT[key] = nc
    return nc


def kernel(**inputs):
    from concourse.bass_utils import run_bass_kernel_spmd

    cfg = full_cfg()
    Bl = cfg["Bl"]
    B = 128
    n_cores = B // Bl

    np_in = {k: np.asarray(v) for k, v in inputs.items()}
    # freeze-snapshot events: union over cores of (freeze step, local column)
    fz_all = np_in["mask"].sum(axis=1).astype(np.int64) - 1
    ev = sorted({(int(fz_all[b]), b % Bl) for b in range(B)})
    cfg = dict(cfg, EV=tuple(ev))
    nc = build_program(cfg, num_devices=n_cores)
    in_maps = []
    for c in range(n_cores):
        sl = slice(c * Bl, (c + 1) * Bl)
        in_maps.append(make_core_inputs(
            cfg,
            np_in["x"][sl], np_in["tags"][sl], np_in["mask"][sl],
            np_in["emb"],
            np_in["Wih_f"], np_in["Whh_f"], np_in["bih_f"], np_in["bhh_f"],
            np_in["Wih_b"], np_in["Whh_b"], np_in["bih_b"], np_in["bhh_b"],
            np_in["W_out"], np_in["b_out"], np_in["transitions"],
            np_in["start_trans"], np_in["end_trans"]))

    res = run_bass_kernel_spmd(nc, in_maps, core_ids=list(range(n_cores)),
                               trace=TRACE)
    if res.exec_time_ns is not None:
        LAST_EXEC_NS.append(res.exec_time_ns)
    vals = np.concatenate([res.results[c]["loss"] for c in range(n_cores)])
    return np.float32(vals.mean())


TRACE = False
LAST_EXEC_NS = []



# revision 6
# speedup vs baseline: 1.0648x; 1.0648x over previous
"""BiLSTM-CRF loss kernel for Trainium2 (8 NeuronCores, data-parallel over batch).

Self-contained: hardcodes shapes B=128, T=512, V=50000, NT=24, E=128, H=256.
Each core processes 16 examples end-to-end.

v2 design (latency-oriented; the kernel is serial-chain bound, not
throughput bound):
  * LSTM: ONE coupled chain per round computing both directions' step
    (one [128,2,4,16] sigmoid, shared cell ops via strided APs).  The
    input projections Wih@x+bias are precomputed chunk-wise directly
    into the PSUM banks that the in-loop Whh@h matmuls then accumulate
    onto (PSUM "pending zero" semantics make this exact).
  * Emissions (W_out@h+b, no exp) and the gold-path reductions are
    interleaved into the LSTM rounds' engine-idle windows.
  * exp() of emissions runs post-LSTM (avoids Act table thrash).
  * CRF: exp-domain, split meet-in-the-middle: alpha recursion over
    positions 0..255 (always unmasked since lengths >= 256) runs
    CONCURRENTLY with the backward (beta/gamma) recursion over
    positions 256..511; per-example end/masking enters gamma via
    exp(end) injections at data-dependent steps.  logZ = ln(sum_i
    alpha_i*gamma_i) + rebase offsets.  Rebasing (overflow control) is
    done OFF the critical chain: scale factors fold into a later
    step's emission operand (legal because the recursion is linear).
"""

import sys

for _p in ("/opt/trn_rl_repo",):
    if _p not in sys.path:
        sys.path.insert(0, _p)

import numpy as np
import ml_dtypes

import concourse.bass as bass
import concourse.bacc as bacc
import concourse.tile as tile
from concourse import mybir
from concourse.bass import IndirectOffsetOnAxis
from concourse.masks import make_identity

F32 = mybir.dt.float32
BF16 = mybir.dt.bfloat16
I32 = mybir.dt.int32
AX = mybir.AxisListType
OP = mybir.AluOpType
ACTF = mybir.ActivationFunctionType


def full_cfg():
    return dict(T=512, Bl=16, V=50000, NT=24, E=128, Hd=128,
                CH=4, RB=4, LAG=2, TG=8)


def build_body(tc, outs, ins, cfg):
    nc = tc.nc
    T, Bl, NT, Hd = cfg["T"], cfg["Bl"], cfg["NT"], cfg["Hd"]
    CH, RB, LAG, TG = cfg["CH"], cfg["RB"], cfg["LAG"], cfg["TG"]
    R = T * Bl
    M = R // 128            # gather tiles (8 positions each)
    NCH = T // CH           # Wx chunks
    IEV = cfg["IEV"]        # sorted distinct lengths in [256, 511]
    NSH = 64                # rebase history slots per chain (63 + terminal)

    import contextlib
    ctx = contextlib.ExitStack()
    with ctx:
        const = ctx.enter_context(tc.tile_pool(name="const", bufs=1))
        big = ctx.enter_context(tc.tile_pool(name="big", bufs=1))

        # ---------------- constants ----------------
        ident = const.tile([128, 128], F32)
        make_identity(nc, ident[:])

        idx_sb = const.tile([128, M], I32)
        nc.sync.dma_start(out=idx_sb[:], in_=ins["idx"][:])
        wih_sb = const.tile([128, 2, 4 * Hd], BF16)
        nc.sync.dma_start(out=wih_sb[:], in_=ins["wih"][:])
        whh_sb = const.tile([128, 2, 4 * Hd], BF16)
        nc.sync.dma_start(out=whh_sb[:], in_=ins["whh"][:])
        wout_sb = const.tile([128, 2, NT], BF16)
        nc.sync.dma_start(out=wout_sb[:], in_=ins["wout"][:])
        bias8_sb = const.tile([1, 8 * 128], F32)
        nc.sync.dma_start(out=bias8_sb[:], in_=ins["bias8"][:])
        bout_sb = const.tile([NT, 1], F32)
        nc.sync.dma_start(out=bout_sb[:], in_=ins["bout"][:])
        trans_sb = const.tile([NT, NT], F32)
        nc.sync.dma_start(out=trans_sb[:], in_=ins["trans"][:])
        Etr_sb = const.tile([NT, NT], F32)
        nc.sync.dma_start(out=Etr_sb[:], in_=ins["Etrans"][:])
        EtrT_sb = const.tile([NT, NT], F32)
        nc.sync.dma_start(out=EtrT_sb[:], in_=ins["EtransT"][:])
        estart_sb = const.tile([NT, 1], F32)
        nc.sync.dma_start(out=estart_sb[:], in_=ins["estart"][:])
        einj_sb = const.tile([1, NT], F32)
        nc.sync.dma_start(out=einj_sb[:], in_=ins["einj"][:])
        injsel_sb = const.tile([1, (len(IEV) + 1) * Bl], F32)
        nc.sync.dma_start(out=injsel_sb[:], in_=ins["injsel"][:])
        selstart_sb = const.tile([NT, Bl], F32)
        nc.sync.dma_start(out=selstart_sb[:], in_=ins["selstart"][:])
        selend_sb = const.tile([NT, Bl], F32)
        nc.sync.dma_start(out=selend_sb[:], in_=ins["selend"][:])
        startv = const.tile([NT, 1], F32)
        nc.sync.dma_start(out=startv[:], in_=ins["startv"][:])
        endv = const.tile([NT, 1], F32)
        nc.sync.dma_start(out=endv[:], in_=ins["endv"][:])

        onesrow = const.tile([1, CH * Bl], F32)
        nc.vector.memset(onesrow[:], 1.0)
        ones1 = const.tile([1, NT], F32)
        nc.vector.memset(ones1[:], 1.0)
        ones24 = const.tile([NT, 1], F32)
        nc.vector.memset(ones24[:], 1.0)

        # ---------------- big persistent tensors ----------------
        xeT = big.tile([128, R], BF16)
        h_f = big.tile([128, R], BF16)
        h_b = big.tile([128, R], BF16)
        emT = big.tile([NT, R], F32)
        EM = big.tile([NT, R], F32)
        cst = big.tile([128, 2, Bl], F32)
        nc.vector.memset(cst[:], 0.0)
        accE = big.tile([NT, Bl], F32)
        nc.vector.memset(accE[:], 0.0)
        accT = big.tile([NT, Bl], F32)
        nc.vector.memset(accT[:], 0.0)
        hista = big.tile([1, NSH * Bl], F32)
        histg = big.tile([1, NSH * Bl], F32)
        qa = big.tile([NT, Bl], F32)

        # ======================= LSTM phase =======================
        with tc.tile_pool(name="wx", bufs=3, space="PSUM") as wxp, \
             tc.tile_pool(name="psA", bufs=1, space="PSUM") as psA, \
             tc.tile_pool(name="psE", bufs=2, space="PSUM") as psE, \
             tc.tile_pool(name="psT", bufs=2, space="PSUM") as psT, \
             tc.tile_pool(name="gath", bufs=3) as gath, \
             tc.tile_pool(name="sgp", bufs=3) as sgp, \
             tc.tile_pool(name="mp", bufs=2) as mp, \
             tc.tile_pool(name="gw", bufs=3) as gw:

            # gather order: ends-first pairs (bwd needs high t, fwd low t)
            gather_order = []
            for i in range((M + 1) // 2):
                gather_order.append(M - 1 - i)
                if i != M - 1 - i:
                    gather_order.append(i)

            def emit_gather(g):
                m = gather_order[g]
                xe_raw = gath.tile([128, 128], F32, tag="xe_raw")
                nc.gpsimd.indirect_dma_start(
                    out=xe_raw[:], out_offset=None,
                    in_=ins["emb"][:],
                    in_offset=IndirectOffsetOnAxis(ap=idx_sb[:, m:m + 1],
                                                   axis=0),
                )
                xe_ps = psA.tile([128, 128], F32, tag="xe_ps")
                nc.tensor.transpose(xe_ps[:], xe_raw[:], ident[:])
                nc.vector.tensor_copy(xeT[:, m * 128:(m + 1) * 128], xe_ps[:])

            wx_tiles = {}

            def emit_precompute(ci):
                wx = wxp.tile([128, 2, 4, CH, Bl], F32, tag="wx")
                wx_tiles[ci] = wx
                t0 = ci * CH
                first = True
                for k in range(4):  # fwd: one matmul per gate, N=CH*Bl
                    nc.tensor.matmul(
                        wx[:, 0, k, :, :],
                        lhsT=wih_sb[:, 0, k * 128:(k + 1) * 128],
                        rhs=xeT[:, t0 * Bl:(t0 + CH) * Bl],
                        start=first, stop=False)
                    first = False
                for k in range(4):  # bwd: per (gate, ri), N=Bl
                    for ri in range(CH):
                        tb = T - 1 - (t0 + ri)
                        nc.tensor.matmul(
                            wx[:, 1, k, ri, :],
                            lhsT=wih_sb[:, 1, k * 128:(k + 1) * 128],
                            rhs=xeT[:, tb * Bl:(tb + 1) * Bl],
                            start=False, stop=False)
                for dk in range(8):  # bias add, K=1
                    nc.tensor.matmul(
                        wx[:, dk // 4, dk % 4, :, :],
                        lhsT=bias8_sb[0:1, dk * 128:(dk + 1) * 128],
                        rhs=onesrow[0:1, :],
                        start=False, stop=False)

            # ---- slack-work emitters ----
            n_ch8 = T // TG     # 64 eight-position chunks

            def emit_emission(k):
                c0, c1 = k * TG * Bl, (k + 1) * TG * Bl
                ps = psE.tile([NT, TG * Bl], F32, tag="em_ps")
                nc.tensor.matmul(ps[:], lhsT=wout_sb[:, 0, :],
                                 rhs=h_f[:, c0:c1], start=True, stop=False)
                nc.tensor.matmul(ps[:], lhsT=wout_sb[:, 1, :],
                                 rhs=h_b[:, c0:c1], start=False, stop=True)
                nc.vector.tensor_scalar_add(emT[:, c0:c1], ps[:], bout_sb[:])

            def emit_gold_em(k):
                c0, c1 = k * TG * Bl, (k + 1) * TG * Bl
                w1 = gw.tile([NT, TG * Bl], F32, tag="w1")
                nc.sync.dma_start(out=w1[:], in_=ins["w1hot"][:, c0:c1])
                mm1 = gw.tile([NT, TG * Bl], F32, tag="mm1")
                nc.vector.tensor_mul(mm1[:], w1[:], emT[:, c0:c1])
                red = gw.tile([NT, Bl], F32, tag="red")
                nc.vector.tensor_reduce(
                    red[:], mm1[:].rearrange("p (t b) -> p b t", b=Bl),
                    axis=AX.X, op=OP.add)
                nc.vector.tensor_add(accE[:], accE[:], red[:])

            tr_tiles = {}

            def emit_trans_dma(j):
                nt_ = min(TG, T - 1 - j * TG)
                c0 = j * TG * Bl
                s1 = gw.tile([NT, TG * Bl], F32, tag="s1")
                nc.sync.dma_start(out=s1[:, 0:nt_ * Bl],
                                  in_=ins["sel1"][:, c0:c0 + nt_ * Bl])
                s2 = gw.tile([NT, TG * Bl], F32, tag="s2")
                nc.sync.dma_start(out=s2[:, 0:nt_ * Bl],
                                  in_=ins["sel2m"][:, c0:c0 + nt_ * Bl])
                tr_tiles[j] = (s1, s2, nt_)

            def emit_gold_trans(j):
                s1, s2, nt_ = tr_tiles.pop(j)
                trp = psT.tile([NT, TG * Bl], F32, tag="trp")
                nc.tensor.matmul(trp[:, 0:nt_ * Bl], lhsT=trans_sb[:],
                                 rhs=s1[:, 0:nt_ * Bl], start=True, stop=True)
                mm2 = gw.tile([NT, TG * Bl], F32, tag="mm2")
                nc.vector.tensor_mul(mm2[:, 0:nt_ * Bl], s2[:, 0:nt_ * Bl],
                                     trp[:, 0:nt_ * Bl])
                red2 = gw.tile([NT, Bl], F32, tag="red2")
                nc.vector.tensor_reduce(
                    red2[:, 0:Bl],
                    mm2[:, 0:nt_ * Bl].rearrange("p (t b) -> p b t", b=Bl),
                    axis=AX.X, op=OP.add)
                nc.vector.tensor_add(accT[:], accT[:], red2[:])

            # schedule bookkeeping
            ready_em = {}
            for k in range(n_ch8):
                rr = max(TG * k + TG - 1, T - 1 - TG * k)
                ready_em.setdefault(rr, []).append(k)
            em_pending = []
            gold_pending = []
            trans_pending = list(range(n_ch8))
            n_tr_dma = 0

            # lead-in: first gathers + chunk 0 Wx
            for g in range(8):
                emit_gather(g)
            next_gather = 8
            emit_precompute(0)

            # ---------------- the coupled recurrence ----------------
            for r in range(T):
                ci, ri = divmod(r, CH)
                wx = wx_tiles[ci]
                if ri == 0 and ci + 1 < NCH:
                    emit_precompute(ci + 1)
                    wx_tiles.pop(ci - 2, None)
                if r > 0:
                    for d in (0, 1):
                        rhs = (h_f[:, (r - 1) * Bl:r * Bl] if d == 0
                               else h_b[:, (T - r) * Bl:(T - r + 1) * Bl])
                        for k in range(4):
                            last = (ri == CH - 1) and d == 1 and k == 3
                            nc.tensor.matmul(
                                wx[:, d, k, ri, :],
                                lhsT=whh_sb[:, d, k * 128:(k + 1) * 128],
                                rhs=rhs, start=False, stop=last)
                sg = sgp.tile([128, 2, 4, Bl], F32, tag="sg")
                nc.scalar.activation(sg[:], wx[:, :, :, ri, :], ACTF.Sigmoid)
                m1 = mp.tile([128, 2, Bl], F32, tag="m1")
                nc.vector.scalar_tensor_tensor(
                    m1[:], sg[:, :, 3, :], 0.5, sg[:, :, 0, :],
                    op0=OP.subtract, op1=OP.mult)
                m2 = mp.tile([128, 2, Bl], F32, tag="m2")
                nc.gpsimd.tensor_mul(m2[:], sg[:, :, 1, :], cst[:])
                nc.vector.scalar_tensor_tensor(
                    cst[:], m1[:], 2.0, m2[:], op0=OP.mult, op1=OP.add)
                tcn = mp.tile([128, 2, Bl], F32, tag="tcn")
                nc.scalar.activation(tcn[:], cst[:], ACTF.Sigmoid, scale=2.0)
                nc.vector.scalar_tensor_tensor(
                    h_f[:, r * Bl:(r + 1) * Bl], tcn[:, 0, :], 0.5,
                    sg[:, 0, 2, :], op0=OP.subtract, op1=OP.mult)
                nc.vector.scalar_tensor_tensor(
                    h_b[:, (T - 1 - r) * Bl:(T - r) * Bl], tcn[:, 1, :], 0.5,
                    sg[:, 1, 2, :], op0=OP.subtract, op1=OP.mult)

                # ---- slack work (one unit per round) ----
                if next_gather < M and r % 3 == 1:
                    emit_gather(next_gather)
                    next_gather += 1
                em_pending.extend(ready_em.get(r, []))
                if n_tr_dma < 2 and r >= 8:
                    emit_trans_dma(n_tr_dma)
                    n_tr_dma += 1
                if em_pending:
                    k = em_pending.pop(0)
                    emit_emission(k)
                    gold_pending.append(k)
                elif gold_pending:
                    emit_gold_em(gold_pending.pop(0))
                elif trans_pending and r >= 16:
                    j = trans_pending.pop(0)
                    emit_gold_trans(j)
                    if n_tr_dma < n_ch8:
                        emit_trans_dma(n_tr_dma)
                        n_tr_dma += 1

            # drain leftovers (a few chunks only become ready at r=T-1)
            for k in em_pending:
                emit_emission(k)
                gold_pending.append(k)
            for k in gold_pending:
                emit_gold_em(k)
            for j in trans_pending:
                if j >= n_tr_dma:
                    emit_trans_dma(j)
                    n_tr_dma = j + 1
                emit_gold_trans(j)

        # ======================= CRF phase =======================
        with tc.tile_pool(name="psCa", bufs=2, space="PSUM") as psCa, \
             tc.tile_pool(name="psCg", bufs=2, space="PSUM") as psCg, \
             tc.tile_pool(name="psOB", bufs=2, space="PSUM") as psOB, \
             tc.tile_pool(name="psD", bufs=1, space="PSUM") as psD, \
             tc.tile_pool(name="cw", bufs=2) as cw, \
             tc.tile_pool(name="cw2", bufs=2) as cw2:

            # exp(emissions), ends-first so both chains start immediately
            exp_order = []
            for i in range(n_ch8 // 2):
                exp_order.extend([n_ch8 - 1 - i, i])
            for k in exp_order:
                c0, c1 = k * TG * Bl, (k + 1) * TG * Bl
                nc.scalar.activation(EM[:, c0:c1], emT[:, c0:c1], ACTF.Exp)

            # alpha init: q0 = exp(start) * EM[:, 0]
            nc.vector.tensor_scalar_mul(qa[:], EM[:, 0:Bl], estart_sb[:])
            # gamma seed: gamma_{T-1} = exp(end) (x) sel{len==T}
            Pg = psCg.tile([NT, Bl], F32, tag="pg")
            nc.tensor.matmul(Pg[:], lhsT=einj_sb[:], rhs=injsel_sb[:, 0:Bl],
                             start=True, stop=True)

            inj_at = {T - 1 - te: e for e, te in enumerate(IEV)}
            emp_a = {}
            emp_g = {}

            def rebase(tag, src_row, hist, slot, em_cols, store):
                """Off-chain rebase: snapshot src_row -> clamp to hist slot,
                reciprocal, broadcast, scale EM[:, em_cols] into store dict."""
                hs = hist[0:1, slot * Bl:(slot + 1) * Bl]
                nc.vector.tensor_scalar_max(hs, src_row, 1.0)
                rc = cw.tile([1, Bl], F32, tag=f"rc{tag}")
                nc.vector.reciprocal(rc[:], hs)
                ob = psOB.tile([NT, Bl], F32, tag="ob")
                nc.tensor.matmul(ob[:], lhsT=ones1[:], rhs=rc[:],
                                 start=True, stop=True)
                if em_cols is None:
                    return ob
                emp = cw2.tile([NT, Bl], F32, tag=f"emp{tag}")
                nc.vector.tensor_mul(emp[:], EM[:, em_cols[0]:em_cols[1]],
                                     ob[:])
                store[em_cols[2]] = emp
                return None

            for s in range(T // 2):
                # gamma step: processes position t, produces gamma_{t-1}
                t = T - 1 - s
                gtmp = cw.tile([NT, Bl], F32, tag="gtmp")
                em_g = emp_g.pop(s, None)
                em_ap = em_g[:] if em_g is not None \
                    else EM[:, t * Bl:(t + 1) * Bl]
                nc.vector.tensor_mul(gtmp[:], em_ap, Pg[:])
                e = inj_at.get(s)
                Pg = psCg.tile([NT, Bl], F32, tag="pg")
                nc.tensor.matmul(Pg[:], lhsT=EtrT_sb[:], rhs=gtmp[:],
                                 start=True, stop=(e is None))
                if e is not None:
                    nc.tensor.matmul(
                        Pg[:], lhsT=einj_sb[:],
                        rhs=injsel_sb[:, (e + 1) * Bl:(e + 2) * Bl],
                        start=False, stop=True)
                # alpha step: position ta
                if s < T // 2 - 1:
                    ta = s + 1
                    Pa = psCa.tile([NT, Bl], F32, tag="pa")
                    nc.tensor.matmul(Pa[:], lhsT=Etr_sb[:], rhs=qa[:],
                                     start=True, stop=True)
                    em_a = emp_a.pop(ta, None)
                    ema_ap = em_a[:] if em_a is not None \
                        else EM[:, ta * Bl:(ta + 1) * Bl]
                    nc.vector.tensor_mul(qa[:], Pa[:], ema_ap)
                    if ta % RB == 0 and ta <= 252:
                        snap = cw.tile([1, Bl], F32, tag="snapa")
                        nc.vector.tensor_mul(snap[:], Pa[0:1, :],
                                             ema_ap[0:1, :])
                        tap = ta + LAG
                        rebase("a", snap[:], hista, ta // RB - 1,
                               (tap * Bl, (tap + 1) * Bl, tap), emp_a)
                if s % RB == 1 and 5 <= s <= 253:
                    tgp = T - 1 - (s + LAG)
                    rebase("g", Pg[0:1, :], histg, (s - 5) // RB,
                           (tgp * Bl, (tgp + 1) * Bl, s + LAG), emp_g)

            # terminal rebases + combine
            oba = rebase("a", qa[0:1, :], hista, NSH - 1, None, None)
            nc.vector.tensor_mul(qa[:], qa[:], oba[:])
            obg = rebase("g", Pg[0:1, :], histg, NSH - 1, None, None)
            nc.vector.tensor_mul(qa[:], qa[:], obg[:])
            de = cw.tile([NT, Bl], F32, tag="de")
            nc.vector.tensor_mul(de[:], qa[:], Pg[:])
            dsum = psD.tile([1, Bl], F32, tag="dsum")
            nc.tensor.matmul(dsum[:], lhsT=ones24[:], rhs=de[:],
                             start=True, stop=True)

            # gold total
            nc.vector.tensor_add(accE[:], accE[:], accT[:])
            gsum = psD.tile([1, Bl], F32, tag="gsum")
            nc.tensor.matmul(gsum[:], lhsT=ones24[:], rhs=accE[:],
                             start=True, stop=False)
            nc.tensor.matmul(gsum[:], lhsT=startv[:], rhs=selstart_sb[:],
                             start=False, stop=False)
            nc.tensor.matmul(gsum[:], lhsT=endv[:], rhs=selend_sb[:],
                             start=False, stop=True)

            # offsets: A = sum_s ln(hist)
            lnA = cw2.tile([1, NSH * Bl], F32, tag="lnA")
            nc.scalar.activation(lnA[:], hista[:], ACTF.Ln)
            Aa = cw.tile([1, Bl], F32, tag="Aa")
            nc.vector.tensor_reduce(
                Aa[:], lnA[:].rearrange("p (s b) -> p b s", b=Bl),
                axis=AX.X, op=OP.add)
            lnG = cw2.tile([1, NSH * Bl], F32, tag="lnG")
            nc.scalar.activation(lnG[:], histg[:], ACTF.Ln)
            Ag = cw.tile([1, Bl], F32, tag="Ag")
            nc.vector.tensor_reduce(
                Ag[:], lnG[:].rearrange("p (s b) -> p b s", b=Bl),
                axis=AX.X, op=OP.add)
            logd = cw.tile([1, Bl], F32, tag="logd")
            nc.scalar.activation(logd[:], dsum[:], ACTF.Ln)

            lr = cw.tile([1, Bl], F32, tag="lr")
            nc.vector.tensor_add(lr[:], logd[:], Aa[:])
            nc.vector.tensor_add(lr[:], lr[:], Ag[:])
            nc.vector.tensor_tensor(lr[:], lr[:], gsum[:], op=OP.subtract)
            nc.sync.dma_start(out=outs["loss"][:].unsqueeze(0), in_=lr[:])


# ======================= host-side preparation =======================

def make_core_inputs(cfg, x, tags, mask, emb, Wih_f, Whh_f, bih_f, bhh_f,
                     Wih_b, Whh_b, bih_b, bhh_b, W_out, b_out,
                     transitions, start_trans, end_trans):
    """Per-core input map (numpy). x/tags/mask are the LOCAL [Bl, T] slices."""
    T, Bl, NT, Hd = cfg["T"], cfg["Bl"], cfg["NT"], cfg["Hd"]
    R = T * Bl
    M = R // 128
    perm = [0, 1, 3, 2]  # torch gate order (i,f,g,o) -> ours (i,f,o,g)

    WIH_S = np.array([1.0, 1.0, 1.0, 2.0], np.float32)   # (i,f,o,g)
    WHH_S = np.array([2.0, 2.0, 2.0, 4.0], np.float32)

    def reorder_rows(w, scales):
        blocks = [w[k * Hd:(k + 1) * Hd] * s for k, s in zip(perm, scales)]
        return np.concatenate(blocks, axis=0)

    def pack_w(wf, wb, scales):
        out = np.empty((128, 2, 4 * Hd), dtype=ml_dtypes.bfloat16)
        out[:, 0, :] = reorder_rows(np.asarray(wf, np.float32), scales).T
        out[:, 1, :] = reorder_rows(np.asarray(wb, np.float32), scales).T
        return out

    def pack_bias(bi, bh):  # -> [4, 128] rows = gates (i,f,o,g)
        b = np.asarray(bi, np.float32) + np.asarray(bh, np.float32)
        return reorder_rows(b, WIH_S).reshape(4, Hd)

    bias8 = np.empty((8, 128), np.float32)
    bias8[0:4] = pack_bias(bih_f, bhh_f)
    bias8[4:8] = pack_bias(bih_b, bhh_b)
    bias8 = bias8.reshape(1, 8 * 128)

    W_out = np.asarray(W_out, np.float32) * 2.0   # h stored as h/2
    wout = np.empty((128, 2, NT), dtype=ml_dtypes.bfloat16)
    wout[:, 0, :] = W_out[:, :Hd].T
    wout[:, 1, :] = W_out[:, Hd:].T

    x = np.asarray(x)
    tags = np.asarray(tags)
    maskf = np.asarray(mask).astype(np.float32)
    trans = np.ascontiguousarray(transitions, np.float32)
    start_trans = np.asarray(start_trans, np.float32)
    end_trans = np.asarray(end_trans, np.float32)

    x_tm = x.T.reshape(-1).astype(np.int32)
    idx = x_tm.reshape(M, 128).T.copy()

    eye = np.eye(NT, dtype=np.float32)
    w = maskf.copy()
    w[:, 0] = 1.0
    w1 = eye[tags]
    w1 = (w1 * w[:, :, None]).transpose(2, 1, 0)
    w1hot = np.ascontiguousarray(w1.reshape(NT, R), np.float32)

    sel1 = eye[tags[:, :-1]].transpose(2, 1, 0).reshape(NT, (T - 1) * Bl)
    sel1 = np.ascontiguousarray(sel1, np.float32)
    sel2 = eye[tags[:, 1:]] * maskf[:, 1:, None]
    sel2m = np.ascontiguousarray(
        sel2.transpose(2, 1, 0).reshape(NT, (T - 1) * Bl), np.float32)

    selstart = np.ascontiguousarray(eye[tags[:, 0]].T, np.float32)
    lens = np.asarray(mask).sum(axis=1).astype(np.int64)
    last_tags = tags[np.arange(Bl), lens - 1]
    selend = np.ascontiguousarray(eye[last_tags].T, np.float32)

    IEV = cfg["IEV"]
    injsel = np.zeros((1, (len(IEV) + 1) * Bl), np.float32)
    injsel[0, 0:Bl] = (lens == T).astype(np.float32)
    for e, te in enumerate(IEV):
        injsel[0, (e + 1) * Bl:(e + 2) * Bl] = \
            (lens == te).astype(np.float32)

    return {
        "emb": np.ascontiguousarray(emb, np.float32),
        "idx": idx,
        "wih": pack_w(Wih_f, Wih_b, WIH_S),
        "whh": pack_w(Whh_f, Whh_b, WHH_S),
        "wout": wout,
        "bias8": bias8,
        "bout": np.asarray(b_out, np.float32).reshape(NT, 1),
        "trans": trans,
        "Etrans": np.exp(trans).astype(np.float32),
        "EtransT": np.ascontiguousarray(np.exp(trans).T, np.float32),
        "estart": np.exp(start_trans).astype(np.float32).reshape(NT, 1),
        "einj": np.exp(end_trans).astype(np.float32).reshape(1, NT),
        "injsel": injsel,
        "selstart": selstart,
        "selend": selend,
        "startv": start_trans.reshape(NT, 1).astype(np.float32),
        "endv": end_trans.reshape(NT, 1).astype(np.float32),
        "w1hot": w1hot,
        "sel1": sel1,
        "sel2m": sel2m,
    }


def input_specs(cfg):
    T, Bl, NT, Hd, V = cfg["T"], cfg["Bl"], cfg["NT"], cfg["Hd"], cfg["V"]
    R = T * Bl
    M = R // 128
    NE = len(cfg["IEV"])
    return {
        "emb": ([V, 128], F32),
        "idx": ([128, M], I32),
        "wih": ([128, 2, 4 * Hd], BF16),
        "whh": ([128, 2, 4 * Hd], BF16),
        "wout": ([128, 2, NT], BF16),
        "bias8": ([1, 8 * 128], F32),
        "bout": ([NT, 1], F32),
        "trans": ([NT, NT], F32),
        "Etrans": ([NT, NT], F32),
        "EtransT": ([NT, NT], F32),
        "estart": ([NT, 1], F32),
        "einj": ([1, NT], F32),
        "injsel": ([1, (NE + 1) * Bl], F32),
        "selstart": ([NT, Bl], F32),
        "selend": ([NT, Bl], F32),
        "startv": ([NT, 1], F32),
        "endv": ([NT, 1], F32),
        "w1hot": ([NT, R], F32),
        "sel1": ([NT, (T - 1) * Bl], F32),
        "sel2m": ([NT, (T - 1) * Bl], F32),
    }


_BUILT = {}


def build_program(cfg, num_devices=8):
    key = tuple((k, v) for k, v in sorted(cfg.items()))
    if key in _BUILT:
        return _BUILT[key]
    nc = bacc.Bacc("TRN2", target_bir_lowering=False, debug=False,
                   num_devices=num_devices)
    ins = {}
    for name, (shape, dt_) in input_specs(cfg).items():
        ins[name] = nc.dram_tensor(name, shape, dt_, kind="ExternalInput").ap()
    outs = {"loss": nc.dram_tensor("loss", [cfg["Bl"]], F32,
                                   kind="ExternalOutput").ap()}
    with tile.TileContext(nc) as tc:
        build_body(tc, outs, ins, cfg)
    nc.compile()
    _BUILT[key] = nc
    return nc


def kernel(**inputs):
    from concourse.bass_utils import run_bass_kernel_spmd

    cfg = full_cfg()
    Bl = cfg["Bl"]
    B = 128
    T = cfg["T"]
    n_cores = B // Bl

    np_in = {k: np.asarray(v) for k, v in inputs.items()}
    lens = np_in["mask"].sum(axis=1).astype(np.int64)
    iev = tuple(sorted({int(l) for l in lens if l < T}))
    cfg = dict(cfg, IEV=iev)
    nc = build_program(cfg, num_devices=n_cores)
    in_maps = []
    for c in range(n_cores):
        sl = slice(c * Bl, (c + 1) * Bl)
        in_maps.append(make_core_inputs(
            cfg,
            np_in["x"][sl], np_in["tags"][sl], np_in["mask"][sl],
            np_in["emb"],
            np_in["Wih_f"], np_in["Whh_f"], np_in["bih_f"], np_in["bhh_f"],
            np_in["Wih_b"], np_in["Whh_b"], np_in["bih_b"], np_in["bhh_b"],
            np_in["W_out"], np_in["b_out"], np_in["transitions"],
            np_in["start_trans"], np_in["end_trans"]))

    res = run_bass_kernel_spmd(nc, in_maps, core_ids=list(range(n_cores)),
                               trace=TRACE)
    if res.exec_time_ns is not None:
        LAST_EXEC_NS.append(res.exec_time_ns)
    vals = np.concatenate([res.results[c]["loss"] for c in range(n_cores)])
    return np.float32(vals.mean())


TRACE = False
LAST_EXEC_NS = []


# revision 7
# speedup vs baseline: 1.1808x; 1.1089x over previous
"""BiLSTM-CRF loss kernel for Trainium2 (8 NeuronCores, data-parallel over batch).

Self-contained: hardcodes shapes B=128, T=512, V=50000, NT=24, E=128, H=256.
Each core processes 16 examples end-to-end.

v2 design (latency-oriented; the kernel is serial-chain bound, not
throughput bound):
  * LSTM: ONE coupled chain per round computing both directions' step
    (one [128,2,4,16] sigmoid, shared cell ops via strided APs).  The
    input projections Wih@x+bias are precomputed chunk-wise directly
    into the PSUM banks that the in-loop Whh@h matmuls then accumulate
    onto (PSUM "pending zero" semantics make this exact).
  * Emissions (W_out@h+b, no exp) and the gold-path reductions are
    interleaved into the LSTM rounds' engine-idle windows.
  * exp() of emissions runs post-LSTM (avoids Act table thrash).
  * CRF: exp-domain, split meet-in-the-middle: alpha recursion over
    positions 0..255 (always unmasked since lengths >= 256) runs
    CONCURRENTLY with the backward (beta/gamma) recursion over
    positions 256..511; per-example end/masking enters gamma via
    exp(end) injections at data-dependent steps.  logZ = ln(sum_i
    alpha_i*gamma_i) + rebase offsets.  Rebasing (overflow control) is
    done OFF the critical chain: scale factors fold into a later
    step's emission operand (legal because the recursion is linear).
"""

import sys

for _p in ("/opt/trn_rl_repo",):
    if _p not in sys.path:
        sys.path.insert(0, _p)

import numpy as np
import ml_dtypes

import concourse.bass as bass
import concourse.bacc as bacc
import concourse.tile as tile
from concourse import mybir
from concourse.bass import IndirectOffsetOnAxis
from concourse.masks import make_identity

F32 = mybir.dt.float32
BF16 = mybir.dt.bfloat16
I32 = mybir.dt.int32
AX = mybir.AxisListType
OP = mybir.AluOpType
ACTF = mybir.ActivationFunctionType


def full_cfg():
    return dict(T=512, Bl=16, V=50000, NT=24, E=128, Hd=128,
                CH=4, RB=4, LAG=2, TG=8)


def build_body(tc, outs, ins, cfg):
    nc = tc.nc
    T, Bl, NT, Hd = cfg["T"], cfg["Bl"], cfg["NT"], cfg["Hd"]
    CH, RB, LAG, TG = cfg["CH"], cfg["RB"], cfg["LAG"], cfg["TG"]
    R = T * Bl
    M = R // 128            # gather tiles (8 positions each)
    NCH = T // CH           # Wx chunks
    IEV = cfg["IEV"]        # sorted distinct lengths in [256, 511]
    NSH = 64                # rebase history slots per chain (63 + terminal)

    import contextlib
    ctx = contextlib.ExitStack()
    with ctx:
        const = ctx.enter_context(tc.tile_pool(name="const", bufs=1))
        big = ctx.enter_context(tc.tile_pool(name="big", bufs=1))

        # ---------------- constants ----------------
        ident = const.tile([128, 128], F32)
        make_identity(nc, ident[:])

        idx_sb = const.tile([128, M], I32)
        nc.sync.dma_start(out=idx_sb[:], in_=ins["idx"][:])
        wih_sb = const.tile([128, 2, 4 * Hd], BF16)
        nc.sync.dma_start(out=wih_sb[:], in_=ins["wih"][:])
        whh_sb = const.tile([128, 2, 4 * Hd], BF16)
        nc.sync.dma_start(out=whh_sb[:], in_=ins["whh"][:])
        wout_sb = const.tile([128, 2, NT], BF16)
        nc.sync.dma_start(out=wout_sb[:], in_=ins["wout"][:])
        bias8_sb = const.tile([1, 8 * 128], F32)
        nc.sync.dma_start(out=bias8_sb[:], in_=ins["bias8"][:])
        bout_sb = const.tile([NT, 1], F32)
        nc.sync.dma_start(out=bout_sb[:], in_=ins["bout"][:])
        trans_sb = const.tile([NT, NT], BF16)
        nc.sync.dma_start(out=trans_sb[:], in_=ins["trans"][:])
        Etr_sb = const.tile([NT, NT], F32)
        nc.sync.dma_start(out=Etr_sb[:], in_=ins["Etrans"][:])
        EtrT_sb = const.tile([NT, NT], F32)
        nc.sync.dma_start(out=EtrT_sb[:], in_=ins["EtransT"][:])
        estart_sb = const.tile([NT, 1], F32)
        nc.sync.dma_start(out=estart_sb[:], in_=ins["estart"][:])
        einj_sb = const.tile([1, NT], F32)
        nc.sync.dma_start(out=einj_sb[:], in_=ins["einj"][:])
        injsel_sb = const.tile([1, (len(IEV) + 1) * Bl], F32)
        nc.sync.dma_start(out=injsel_sb[:], in_=ins["injsel"][:])
        selstart_sb = const.tile([NT, Bl], F32)
        nc.sync.dma_start(out=selstart_sb[:], in_=ins["selstart"][:])
        selend_sb = const.tile([NT, Bl], F32)
        nc.sync.dma_start(out=selend_sb[:], in_=ins["selend"][:])
        startv = const.tile([NT, 1], F32)
        nc.sync.dma_start(out=startv[:], in_=ins["startv"][:])
        endv = const.tile([NT, 1], F32)
        nc.sync.dma_start(out=endv[:], in_=ins["endv"][:])

        onesrow = const.tile([1, CH * Bl], F32)
        nc.vector.memset(onesrow[:], 1.0)
        ones1 = const.tile([1, NT], F32)
        nc.vector.memset(ones1[:], 1.0)
        ones24 = const.tile([NT, 1], F32)
        nc.vector.memset(ones24[:], 1.0)

        # ---------------- big persistent tensors ----------------
        xeT = big.tile([128, R], BF16)
        h_f = big.tile([128, R], BF16)
        h_b = big.tile([128, R], BF16)
        emT = big.tile([NT, R], F32)
        EM = big.tile([NT, R], F32)
        cst = big.tile([128, 2, Bl], F32)
        nc.vector.memset(cst[:], 0.0)
        accE = big.tile([NT, Bl], F32)
        nc.vector.memset(accE[:], 0.0)
        accT = big.tile([NT, Bl], F32)
        nc.vector.memset(accT[:], 0.0)
        w1_sb = big.tile([NT, R], BF16)
        nc.sync.dma_start(out=w1_sb[:], in_=ins["w1hot"][:])
        s1_sb = big.tile([NT, (T - 1) * Bl], BF16)
        nc.sync.dma_start(out=s1_sb[:], in_=ins["sel1"][:])
        s2_sb = big.tile([NT, (T - 1) * Bl], BF16)
        nc.sync.dma_start(out=s2_sb[:], in_=ins["sel2m"][:])
        hista = big.tile([1, NSH * Bl], F32)
        histg = big.tile([1, NSH * Bl], F32)
        qa = big.tile([NT, Bl], F32)

        # ======================= LSTM phase =======================
        with tc.tile_pool(name="wx", bufs=3, space="PSUM") as wxp, \
             tc.tile_pool(name="psA", bufs=1, space="PSUM") as psA, \
             tc.tile_pool(name="psE", bufs=2, space="PSUM") as psE, \
             tc.tile_pool(name="psT", bufs=2, space="PSUM") as psT, \
             tc.tile_pool(name="gath", bufs=3) as gath, \
             tc.tile_pool(name="sgp", bufs=3) as sgp, \
             tc.tile_pool(name="mp", bufs=2) as mp, \
             tc.tile_pool(name="gw", bufs=3) as gw:

            # gather order: ends-first pairs (bwd needs high t, fwd low t)
            gather_order = []
            for i in range((M + 1) // 2):
                gather_order.append(M - 1 - i)
                if i != M - 1 - i:
                    gather_order.append(i)

            def emit_gather(g):
                m = gather_order[g]
                xe_raw = gath.tile([128, 128], F32, tag="xe_raw")
                nc.gpsimd.indirect_dma_start(
                    out=xe_raw[:], out_offset=None,
                    in_=ins["emb"][:],
                    in_offset=IndirectOffsetOnAxis(ap=idx_sb[:, m:m + 1],
                                                   axis=0),
                )
                xe_ps = psA.tile([128, 128], F32, tag="xe_ps")
                nc.tensor.transpose(xe_ps[:], xe_raw[:], ident[:])
                nc.vector.tensor_copy(xeT[:, m * 128:(m + 1) * 128], xe_ps[:])

            wx_tiles = {}

            def emit_precompute(ci):
                wx = wxp.tile([128, 2, 4, CH, Bl], F32, tag="wx")
                wx_tiles[ci] = wx
                t0 = ci * CH
                first = True
                for k in range(4):  # fwd: one matmul per gate, N=CH*Bl
                    nc.tensor.matmul(
                        wx[:, 0, k, :, :],
                        lhsT=wih_sb[:, 0, k * 128:(k + 1) * 128],
                        rhs=xeT[:, t0 * Bl:(t0 + CH) * Bl],
                        start=first, stop=False)
                    first = False
                for k in range(4):  # bwd: per (gate, ri), N=Bl
                    for ri in range(CH):
                        tb = T - 1 - (t0 + ri)
                        nc.tensor.matmul(
                            wx[:, 1, k, ri, :],
                            lhsT=wih_sb[:, 1, k * 128:(k + 1) * 128],
                            rhs=xeT[:, tb * Bl:(tb + 1) * Bl],
                            start=False, stop=False)
                for dk in range(8):  # bias add, K=1
                    nc.tensor.matmul(
                        wx[:, dk // 4, dk % 4, :, :],
                        lhsT=bias8_sb[0:1, dk * 128:(dk + 1) * 128],
                        rhs=onesrow[0:1, :],
                        start=False, stop=False)

            # ---- slack-work emitters ----
            n_ch8 = T // TG     # 64 eight-position chunks

            def emit_emission(k):
                c0, c1 = k * TG * Bl, (k + 1) * TG * Bl
                ps = psE.tile([NT, TG * Bl], F32, tag="em_ps")
                nc.tensor.matmul(ps[:], lhsT=wout_sb[:, 0, :],
                                 rhs=h_f[:, c0:c1], start=True, stop=False)
                nc.tensor.matmul(ps[:], lhsT=wout_sb[:, 1, :],
                                 rhs=h_b[:, c0:c1], start=False, stop=True)
                nc.vector.tensor_scalar_add(emT[:, c0:c1], ps[:], bout_sb[:])

            def emit_gold_em(k):
                c0, c1 = k * TG * Bl, (k + 1) * TG * Bl
                mm1 = gw.tile([NT, TG * Bl], F32, tag="mm1")
                nc.vector.tensor_mul(mm1[:], w1_sb[:, c0:c1], emT[:, c0:c1])
                red = gw.tile([NT, Bl], F32, tag="red")
                nc.vector.tensor_reduce(
                    red[:], mm1[:].rearrange("p (t b) -> p b t", b=Bl),
                    axis=AX.X, op=OP.add)
                nc.vector.tensor_add(accE[:], accE[:], red[:])

            def emit_gold_trans(j):
                nt_ = min(TG, T - 1 - j * TG)
                c0 = j * TG * Bl
                trp = psT.tile([NT, TG * Bl], F32, tag="trp")
                nc.tensor.matmul(trp[:, 0:nt_ * Bl], lhsT=trans_sb[:],
                                 rhs=s1_sb[:, c0:c0 + nt_ * Bl],
                                 start=True, stop=True)
                mm2 = gw.tile([NT, TG * Bl], F32, tag="mm2")
                nc.vector.tensor_mul(mm2[:, 0:nt_ * Bl],
                                     s2_sb[:, c0:c0 + nt_ * Bl],
                                     trp[:, 0:nt_ * Bl])
                red2 = gw.tile([NT, Bl], F32, tag="red2")
                nc.vector.tensor_reduce(
                    red2[:, 0:Bl],
                    mm2[:, 0:nt_ * Bl].rearrange("p (t b) -> p b t", b=Bl),
                    axis=AX.X, op=OP.add)
                nc.vector.tensor_add(accT[:], accT[:], red2[:])

            # lead-in: first gathers + chunk 0 Wx
            for g in range(8):
                emit_gather(g)
            next_gather = 8
            emit_precompute(0)

            # ---------------- the coupled recurrence ----------------
            for r in range(T):
                ci, ri = divmod(r, CH)
                wx = wx_tiles[ci]
                if ri == 0 and ci + 1 < NCH:
                    emit_precompute(ci + 1)
                    wx_tiles.pop(ci - 2, None)
                if r > 0:
                    for d in (0, 1):
                        rhs = (h_f[:, (r - 1) * Bl:r * Bl] if d == 0
                               else h_b[:, (T - r) * Bl:(T - r + 1) * Bl])
                        for k in range(4):
                            last = (ri == CH - 1) and d == 1 and k == 3
                            nc.tensor.matmul(
                                wx[:, d, k, ri, :],
                                lhsT=whh_sb[:, d, k * 128:(k + 1) * 128],
                                rhs=rhs, start=False, stop=last)
                sg = sgp.tile([128, 2, 4, Bl], F32, tag="sg")
                nc.scalar.activation(sg[:], wx[:, :, :, ri, :], ACTF.Sigmoid)
                m1 = mp.tile([128, 2, Bl], F32, tag="m1")
                nc.vector.scalar_tensor_tensor(
                    m1[:], sg[:, :, 3, :], 0.5, sg[:, :, 0, :],
                    op0=OP.subtract, op1=OP.mult)
                m2 = mp.tile([128, 2, Bl], F32, tag="m2")
                nc.gpsimd.tensor_mul(m2[:], sg[:, :, 1, :], cst[:])
                nc.vector.scalar_tensor_tensor(
                    cst[:], m1[:], 2.0, m2[:], op0=OP.mult, op1=OP.add)
                tcn = mp.tile([128, 2, Bl], F32, tag="tcn")
                nc.scalar.activation(tcn[:], cst[:], ACTF.Sigmoid, scale=2.0)
                nc.vector.scalar_tensor_tensor(
                    h_f[:, r * Bl:(r + 1) * Bl], tcn[:, 0, :], 0.5,
                    sg[:, 0, 2, :], op0=OP.subtract, op1=OP.mult)
                nc.vector.scalar_tensor_tensor(
                    h_b[:, (T - 1 - r) * Bl:(T - r) * Bl], tcn[:, 1, :], 0.5,
                    sg[:, 1, 2, :], op0=OP.subtract, op1=OP.mult)

                # ---- gathers feed the chain; keep them paced in-loop ----
                if next_gather < M and r % 3 == 1:
                    emit_gather(next_gather)
                    next_gather += 1

            # emissions + gold: emitted after the loop in readiness order;
            # the list scheduler backfills them into engine-idle windows.
            for k in sorted(range(n_ch8),
                            key=lambda k: max(TG * k + TG - 1, T - 1 - TG * k)):
                emit_emission(k)
                emit_gold_em(k)
            for j in range(n_ch8):
                emit_gold_trans(j)

        # ======================= CRF phase =======================
        with tc.tile_pool(name="psCa", bufs=2, space="PSUM") as psCa, \
             tc.tile_pool(name="psCg", bufs=2, space="PSUM") as psCg, \
             tc.tile_pool(name="psOB", bufs=2, space="PSUM") as psOB, \
             tc.tile_pool(name="psD", bufs=1, space="PSUM") as psD, \
             tc.tile_pool(name="cw", bufs=2) as cw, \
             tc.tile_pool(name="cw2", bufs=2) as cw2:

            # token written only after the last LSTM round: gates every Exp
            # op so the scheduler cannot hoist them into the Sigmoid loop
            # (each hoist would cost 2x1283ns act-table loads on the chain).
            tok = cw.tile([NT, 1], F32, tag="tok")
            nc.vector.tensor_scalar_mul(tok[:], h_b[0:NT, 0:1], 0.0)
            # exp(emissions), ends-first so both chains start immediately
            exp_order = []
            for i in range(n_ch8 // 2):
                exp_order.extend([n_ch8 - 1 - i, i])
            for k in exp_order:
                c0, c1 = k * TG * Bl, (k + 1) * TG * Bl
                nc.scalar.activation(EM[:, c0:c1], emT[:, c0:c1], ACTF.Exp,
                                     bias=tok[:])

            # alpha init: q0 = exp(start) * EM[:, 0]
            nc.vector.tensor_scalar_mul(qa[:], EM[:, 0:Bl], estart_sb[:])
            # gamma seed: gamma_{T-1} = exp(end) (x) sel{len==T}
            Pg = psCg.tile([NT, Bl], F32, tag="pg")
            nc.tensor.matmul(Pg[:], lhsT=einj_sb[:], rhs=injsel_sb[:, 0:Bl],
                             start=True, stop=True)

            inj_at = {T - 1 - te: e for e, te in enumerate(IEV)}
            emp_a = {}
            emp_g = {}

            def rebase(tag, src_row, hist, slot, em_cols, store):
                """Off-chain rebase: snapshot src_row -> clamp to hist slot,
                reciprocal, broadcast, scale EM[:, em_cols] into store dict."""
                hs = hist[0:1, slot * Bl:(slot + 1) * Bl]
                nc.vector.tensor_scalar_max(hs, src_row, 1.0)
                rc = cw.tile([1, Bl], F32, tag=f"rc{tag}")
                nc.vector.reciprocal(rc[:], hs)
                ob = psOB.tile([NT, Bl], F32, tag="ob")
                nc.tensor.matmul(ob[:], lhsT=ones1[:], rhs=rc[:],
                                 start=True, stop=True)
                if em_cols is None:
                    return ob
                emp = cw2.tile([NT, Bl], F32, tag=f"emp{tag}")
                nc.vector.tensor_mul(emp[:], EM[:, em_cols[0]:em_cols[1]],
                                     ob[:])
                store[em_cols[2]] = emp
                return None

            for s in range(T // 2):
                # gamma step: processes position t, produces gamma_{t-1}
                t = T - 1 - s
                gtmp = cw.tile([NT, Bl], F32, tag="gtmp")
                em_g = emp_g.pop(s, None)
                em_ap = em_g[:] if em_g is not None \
                    else EM[:, t * Bl:(t + 1) * Bl]
                nc.vector.tensor_mul(gtmp[:], em_ap, Pg[:])
                e = inj_at.get(s)
                Pg = psCg.tile([NT, Bl], F32, tag="pg")
                nc.tensor.matmul(Pg[:], lhsT=EtrT_sb[:], rhs=gtmp[:],
                                 start=True, stop=(e is None))
                if e is not None:
                    nc.tensor.matmul(
                        Pg[:], lhsT=einj_sb[:],
                        rhs=injsel_sb[:, (e + 1) * Bl:(e + 2) * Bl],
                        start=False, stop=True)
                # alpha step: position ta
                if s < T // 2 - 1:
                    ta = s + 1
                    Pa = psCa.tile([NT, Bl], F32, tag="pa")
                    nc.tensor.matmul(Pa[:], lhsT=Etr_sb[:], rhs=qa[:],
                                     start=True, stop=True)
                    em_a = emp_a.pop(ta, None)
                    ema_ap = em_a[:] if em_a is not None \
                        else EM[:, ta * Bl:(ta + 1) * Bl]
                    nc.vector.tensor_mul(qa[:], Pa[:], ema_ap)
                    if ta % RB == 0 and ta <= 252:
                        snap = cw.tile([1, Bl], F32, tag="snapa")
                        nc.vector.tensor_mul(snap[:], Pa[0:1, :],
                                             ema_ap[0:1, :])
                        tap = ta + LAG
                        rebase("a", snap[:], hista, ta // RB - 1,
                               (tap * Bl, (tap + 1) * Bl, tap), emp_a)
                if s % RB == 1 and 5 <= s <= 253:
                    tgp = T - 1 - (s + LAG)
                    rebase("g", Pg[0:1, :], histg, (s - 5) // RB,
                           (tgp * Bl, (tgp + 1) * Bl, s + LAG), emp_g)

            # terminal rebases + combine
            oba = rebase("a", qa[0:1, :], hista, NSH - 1, None, None)
            nc.vector.tensor_mul(qa[:], qa[:], oba[:])
            obg = rebase("g", Pg[0:1, :], histg, NSH - 1, None, None)
            nc.vector.tensor_mul(qa[:], qa[:], obg[:])
            de = cw.tile([NT, Bl], F32, tag="de")
            nc.vector.tensor_mul(de[:], qa[:], Pg[:])
            dsum = psD.tile([1, Bl], F32, tag="dsum")
            nc.tensor.matmul(dsum[:], lhsT=ones24[:], rhs=de[:],
                             start=True, stop=True)

            # gold total
            nc.vector.tensor_add(accE[:], accE[:], accT[:])
            gsum = psD.tile([1, Bl], F32, tag="gsum")
            nc.tensor.matmul(gsum[:], lhsT=ones24[:], rhs=accE[:],
                             start=True, stop=False)
            nc.tensor.matmul(gsum[:], lhsT=startv[:], rhs=selstart_sb[:],
                             start=False, stop=False)
            nc.tensor.matmul(gsum[:], lhsT=endv[:], rhs=selend_sb[:],
                             start=False, stop=True)

            # offsets: A = sum_s ln(hist)
            lnA = cw2.tile([1, NSH * Bl], F32, tag="lnA")
            nc.scalar.activation(lnA[:], hista[:], ACTF.Ln)
            Aa = cw.tile([1, Bl], F32, tag="Aa")
            nc.vector.tensor_reduce(
                Aa[:], lnA[:].rearrange("p (s b) -> p b s", b=Bl),
                axis=AX.X, op=OP.add)
            lnG = cw2.tile([1, NSH * Bl], F32, tag="lnG")
            nc.scalar.activation(lnG[:], histg[:], ACTF.Ln)
            Ag = cw.tile([1, Bl], F32, tag="Ag")
            nc.vector.tensor_reduce(
                Ag[:], lnG[:].rearrange("p (s b) -> p b s", b=Bl),
                axis=AX.X, op=OP.add)
            logd = cw.tile([1, Bl], F32, tag="logd")
            nc.scalar.activation(logd[:], dsum[:], ACTF.Ln)

            lr = cw.tile([1, Bl], F32, tag="lr")
            nc.vector.tensor_add(lr[:], logd[:], Aa[:])
            nc.vector.tensor_add(lr[:], lr[:], Ag[:])
            nc.vector.tensor_tensor(lr[:], lr[:], gsum[:], op=OP.subtract)
            nc.sync.dma_start(out=outs["loss"][:].unsqueeze(0), in_=lr[:])


# ======================= host-side preparation =======================

def make_core_inputs(cfg, x, tags, mask, emb, Wih_f, Whh_f, bih_f, bhh_f,
                     Wih_b, Whh_b, bih_b, bhh_b, W_out, b_out,
                     transitions, start_trans, end_trans):
    """Per-core input map (numpy). x/tags/mask are the LOCAL [Bl, T] slices."""
    T, Bl, NT, Hd = cfg["T"], cfg["Bl"], cfg["NT"], cfg["Hd"]
    R = T * Bl
    M = R // 128
    perm = [0, 1, 3, 2]  # torch gate order (i,f,g,o) -> ours (i,f,o,g)

    WIH_S = np.array([1.0, 1.0, 1.0, 2.0], np.float32)   # (i,f,o,g)
    WHH_S = np.array([2.0, 2.0, 2.0, 4.0], np.float32)

    def reorder_rows(w, scales):
        blocks = [w[k * Hd:(k + 1) * Hd] * s for k, s in zip(perm, scales)]
        return np.concatenate(blocks, axis=0)

    def pack_w(wf, wb, scales):
        out = np.empty((128, 2, 4 * Hd), dtype=ml_dtypes.bfloat16)
        out[:, 0, :] = reorder_rows(np.asarray(wf, np.float32), scales).T
        out[:, 1, :] = reorder_rows(np.asarray(wb, np.float32), scales).T
        return out

    def pack_bias(bi, bh):  # -> [4, 128] rows = gates (i,f,o,g)
        b = np.asarray(bi, np.float32) + np.asarray(bh, np.float32)
        return reorder_rows(b, WIH_S).reshape(4, Hd)

    bias8 = np.empty((8, 128), np.float32)
    bias8[0:4] = pack_bias(bih_f, bhh_f)
    bias8[4:8] = pack_bias(bih_b, bhh_b)
    bias8 = bias8.reshape(1, 8 * 128)

    W_out = np.asarray(W_out, np.float32) * 2.0   # h stored as h/2
    wout = np.empty((128, 2, NT), dtype=ml_dtypes.bfloat16)
    wout[:, 0, :] = W_out[:, :Hd].T
    wout[:, 1, :] = W_out[:, Hd:].T

    x = np.asarray(x)
    tags = np.asarray(tags)
    maskf = np.asarray(mask).astype(np.float32)
    trans = np.ascontiguousarray(transitions, np.float32)
    transb = trans.astype(ml_dtypes.bfloat16)
    start_trans = np.asarray(start_trans, np.float32)
    end_trans = np.asarray(end_trans, np.float32)

    x_tm = x.T.reshape(-1).astype(np.int32)
    idx = x_tm.reshape(M, 128).T.copy()

    eye = np.eye(NT, dtype=np.float32)
    w = maskf.copy()
    w[:, 0] = 1.0
    w1 = eye[tags]
    w1 = (w1 * w[:, :, None]).transpose(2, 1, 0)
    w1hot = np.ascontiguousarray(w1.reshape(NT, R), np.float32)

    sel1 = eye[tags[:, :-1]].transpose(2, 1, 0).reshape(NT, (T - 1) * Bl)
    sel1 = np.ascontiguousarray(sel1, np.float32)
    sel2 = eye[tags[:, 1:]] * maskf[:, 1:, None]
    sel2m = np.ascontiguousarray(
        sel2.transpose(2, 1, 0).reshape(NT, (T - 1) * Bl), np.float32)

    selstart = np.ascontiguousarray(eye[tags[:, 0]].T, np.float32)
    lens = np.asarray(mask).sum(axis=1).astype(np.int64)
    last_tags = tags[np.arange(Bl), lens - 1]
    selend = np.ascontiguousarray(eye[last_tags].T, np.float32)

    IEV = cfg["IEV"]
    injsel = np.zeros((1, (len(IEV) + 1) * Bl), np.float32)
    injsel[0, 0:Bl] = (lens == T).astype(np.float32)
    for e, te in enumerate(IEV):
        injsel[0, (e + 1) * Bl:(e + 2) * Bl] = \
            (lens == te).astype(np.float32)

    return {
        "emb": np.ascontiguousarray(emb, np.float32),
        "idx": idx,
        "wih": pack_w(Wih_f, Wih_b, WIH_S),
        "whh": pack_w(Whh_f, Whh_b, WHH_S),
        "wout": wout,
        "bias8": bias8,
        "bout": np.asarray(b_out, np.float32).reshape(NT, 1),
        "trans": transb,
        "Etrans": np.exp(trans).astype(np.float32),
        "EtransT": np.ascontiguousarray(np.exp(trans).T, np.float32),
        "estart": np.exp(start_trans).astype(np.float32).reshape(NT, 1),
        "einj": np.exp(end_trans).astype(np.float32).reshape(1, NT),
        "injsel": injsel,
        "selstart": selstart,
        "selend": selend,
        "startv": start_trans.reshape(NT, 1).astype(np.float32),
        "endv": end_trans.reshape(NT, 1).astype(np.float32),
        "w1hot": w1hot.astype(ml_dtypes.bfloat16),
        "sel1": sel1.astype(ml_dtypes.bfloat16),
        "sel2m": sel2m.astype(ml_dtypes.bfloat16),
    }


def input_specs(cfg):
    T, Bl, NT, Hd, V = cfg["T"], cfg["Bl"], cfg["NT"], cfg["Hd"], cfg["V"]
    R = T * Bl
    M = R // 128
    NE = len(cfg["IEV"])
    return {
        "emb": ([V, 128], F32),
        "idx": ([128, M], I32),
        "wih": ([128, 2, 4 * Hd], BF16),
        "whh": ([128, 2, 4 * Hd], BF16),
        "wout": ([128, 2, NT], BF16),
        "bias8": ([1, 8 * 128], F32),
        "bout": ([NT, 1], F32),
        "trans": ([NT, NT], BF16),
        "Etrans": ([NT, NT], F32),
        "EtransT": ([NT, NT], F32),
        "estart": ([NT, 1], F32),
        "einj": ([1, NT], F32),
        "injsel": ([1, (NE + 1) * Bl], F32),
        "selstart": ([NT, Bl], F32),
        "selend": ([NT, Bl], F32),
        "startv": ([NT, 1], F32),
        "endv": ([NT, 1], F32),
        "w1hot": ([NT, R], BF16),
        "sel1": ([NT, (T - 1) * Bl], BF16),
        "sel2m": ([NT, (T - 1) * Bl], BF16),
    }


_BUILT = {}


def build_program(cfg, num_devices=8):
    key = tuple((k, v) for k, v in sorted(cfg.items()))
    if key in _BUILT:
        return _BUILT[key]
    nc = bacc.Bacc("TRN2", target_bir_lowering=False, debug=False,
                   num_devices=num_devices)
    ins = {}
    for name, (shape, dt_) in input_specs(cfg).items():
        ins[name] = nc.dram_tensor(name, shape, dt_, kind="ExternalInput").ap()
    outs = {"loss": nc.dram_tensor("loss", [cfg["Bl"]], F32,
                                   kind="ExternalOutput").ap()}
    with tile.TileContext(nc) as tc:
        build_body(tc, outs, ins, cfg)
    nc.compile()
    _BUILT[key] = nc
    return nc


def kernel(**inputs):
    from concourse.bass_utils import run_bass_kernel_spmd

    cfg = full_cfg()
    Bl = cfg["Bl"]
    B = 128
    T = cfg["T"]
    n_cores = B // Bl

    np_in = {k: np.asarray(v) for k, v in inputs.items()}
    lens = np_in["mask"].sum(axis=1).astype(np.int64)
    iev = tuple(sorted({int(l) for l in lens if l < T}))
    cfg = dict(cfg, IEV=iev)
    nc = build_program(cfg, num_devices=n_cores)
    in_maps = []
    for c in range(n_cores):
        sl = slice(c * Bl, (c + 1) * Bl)
        in_maps.append(make_core_inputs(
            cfg,
            np_in["x"][sl], np_in["tags"][sl], np_in["mask"][sl],
            np_in["emb"],
            np_in["Wih_f"], np_in["Whh_f"], np_in["bih_f"], np_in["bhh_f"],
            np_in["Wih_b"], np_in["Whh_b"], np_in["bih_b"], np_in["bhh_b"],
            np_in["W_out"], np_in["b_out"], np_in["transitions"],
            np_in["start_trans"], np_in["end_trans"]))

    res = run_bass_kernel_spmd(nc, in_maps, core_ids=list(range(n_cores)),
                               trace=TRACE)
    if res.exec_time_ns is not None:
        LAST_EXEC_NS.append(res.exec_time_ns)
    vals = np.concatenate([res.results[c]["loss"] for c in range(n_cores)])
    return np.float32(vals.mean())


TRACE = False
LAST_EXEC_NS = []


# revision 9
# speedup vs baseline: 1.1941x; 1.0113x over previous
"""BiLSTM-CRF loss kernel for Trainium2 (8 NeuronCores, data-parallel over batch).

Self-contained: hardcodes shapes B=128, T=512, V=50000, NT=24, E=128, H=256.
Each core processes 16 examples end-to-end.

v2 design (latency-oriented; the kernel is serial-chain bound, not
throughput bound):
  * LSTM: ONE coupled chain per round computing both directions' step
    (one [128,2,4,16] sigmoid, shared cell ops via strided APs).  The
    input projections Wih@x+bias are precomputed chunk-wise directly
    into the PSUM banks that the in-loop Whh@h matmuls then accumulate
    onto (PSUM "pending zero" semantics make this exact).
  * Emissions (W_out@h+b, no exp) and the gold-path reductions are
    interleaved into the LSTM rounds' engine-idle windows.
  * exp() of emissions runs post-LSTM (avoids Act table thrash).
  * CRF: exp-domain, split meet-in-the-middle: alpha recursion over
    positions 0..255 (always unmasked since lengths >= 256) runs
    CONCURRENTLY with the backward (beta/gamma) recursion over
    positions 256..511; per-example end/masking enters gamma via
    exp(end) injections at data-dependent steps.  logZ = ln(sum_i
    alpha_i*gamma_i) + rebase offsets.  Rebasing (overflow control) is
    done OFF the critical chain: scale factors fold into a later
    step's emission operand (legal because the recursion is linear).
"""

import sys

for _p in ("/opt/trn_rl_repo",):
    if _p not in sys.path:
        sys.path.insert(0, _p)

import numpy as np
import ml_dtypes

import concourse.bass as bass
import concourse.bacc as bacc
import concourse.tile as tile
from concourse import mybir
from concourse.bass import IndirectOffsetOnAxis
from concourse.masks import make_identity

F32 = mybir.dt.float32
BF16 = mybir.dt.bfloat16
I32 = mybir.dt.int32
AX = mybir.AxisListType
OP = mybir.AluOpType
ACTF = mybir.ActivationFunctionType


def full_cfg():
    return dict(T=512, Bl=16, V=50000, NT=24, E=128, Hd=128,
                CH=4, RB=4, LAG=2, TG=8)


def build_body(tc, outs, ins, cfg):
    nc = tc.nc
    T, Bl, NT, Hd = cfg["T"], cfg["Bl"], cfg["NT"], cfg["Hd"]
    CH, RB, LAG, TG = cfg["CH"], cfg["RB"], cfg["LAG"], cfg["TG"]
    R = T * Bl
    M = R // 128            # gather tiles (8 positions each)
    NCH = T // CH           # Wx chunks
    IEV = cfg["IEV"]        # sorted distinct lengths in [256, 511]
    NSH = 64                # rebase history slots per chain

    import contextlib
    ctx = contextlib.ExitStack()
    with ctx:
        const = ctx.enter_context(tc.tile_pool(name="const", bufs=1))
        big = ctx.enter_context(tc.tile_pool(name="big", bufs=1))

        # ---------------- constants ----------------
        ident = const.tile([128, 128], F32)
        make_identity(nc, ident[:])

        idx_sb = const.tile([128, M], I32)
        nc.sync.dma_start(out=idx_sb[:], in_=ins["idx"][:])
        wih_sb = const.tile([128, 2, 4 * Hd], BF16)
        nc.sync.dma_start(out=wih_sb[:], in_=ins["wih"][:])
        whh_sb = const.tile([128, 2, 4 * Hd], BF16)
        nc.sync.dma_start(out=whh_sb[:], in_=ins["whh"][:])
        wout_sb = const.tile([128, 2, NT], BF16)
        nc.sync.dma_start(out=wout_sb[:], in_=ins["wout"][:])
        bias8_sb = const.tile([1, 8 * 128], F32)
        nc.sync.dma_start(out=bias8_sb[:], in_=ins["bias8"][:])
        bout_sb = const.tile([NT, 1], F32)
        nc.sync.dma_start(out=bout_sb[:], in_=ins["bout"][:])
        trans_sb = const.tile([NT, NT], BF16)
        nc.sync.dma_start(out=trans_sb[:], in_=ins["trans"][:])
        Etr_sb = const.tile([NT, NT], F32)
        nc.sync.dma_start(out=Etr_sb[:], in_=ins["Etrans"][:])
        EtrT_sb = const.tile([NT, NT], F32)
        nc.sync.dma_start(out=EtrT_sb[:], in_=ins["EtransT"][:])
        estart_sb = const.tile([NT, 1], F32)
        nc.sync.dma_start(out=estart_sb[:], in_=ins["estart"][:])
        einj_sb = const.tile([1, NT], F32)
        nc.sync.dma_start(out=einj_sb[:], in_=ins["einj"][:])
        injsel_sb = const.tile([1, (len(IEV) + 1) * Bl], F32)
        nc.sync.dma_start(out=injsel_sb[:], in_=ins["injsel"][:])
        selstart_sb = const.tile([NT, Bl], F32)
        nc.sync.dma_start(out=selstart_sb[:], in_=ins["selstart"][:])
        selend_sb = const.tile([NT, Bl], F32)
        nc.sync.dma_start(out=selend_sb[:], in_=ins["selend"][:])
        startv = const.tile([NT, 1], F32)
        nc.sync.dma_start(out=startv[:], in_=ins["startv"][:])
        endv = const.tile([NT, 1], F32)
        nc.sync.dma_start(out=endv[:], in_=ins["endv"][:])

        onesrow = const.tile([1, CH * Bl], F32)
        nc.vector.memset(onesrow[:], 1.0)
        ones1 = const.tile([1, NT], F32)
        nc.vector.memset(ones1[:], 1.0)
        ones24 = const.tile([NT, 1], F32)
        nc.vector.memset(ones24[:], 1.0)

        # ---------------- big persistent tensors ----------------
        xeT = big.tile([128, R], BF16)
        h_f = big.tile([128, R], BF16)
        h_b = big.tile([128, R], BF16)
        emT = big.tile([NT, R], F32)
        EM = big.tile([NT, R], F32)
        cst = big.tile([128, 2, Bl], F32)
        nc.vector.memset(cst[:], 0.0)
        accE = big.tile([NT, Bl], F32)
        nc.vector.memset(accE[:], 0.0)
        accT = big.tile([NT, Bl], F32)
        nc.vector.memset(accT[:], 0.0)
        w1_sb = big.tile([NT, R], BF16)
        nc.sync.dma_start(out=w1_sb[:], in_=ins["w1hot"][:])
        s1_sb = big.tile([NT, (T - 1) * Bl], BF16)
        nc.sync.dma_start(out=s1_sb[:], in_=ins["sel1"][:])
        s2_sb = big.tile([NT, (T - 1) * Bl], BF16)
        nc.sync.dma_start(out=s2_sb[:], in_=ins["sel2m"][:])
        hista = big.tile([1, NSH * Bl], F32)
        nc.vector.memset(hista[:], 1.0)
        histg = big.tile([1, NSH * Bl], F32)
        nc.vector.memset(histg[:], 1.0)
        qa = big.tile([NT, Bl], F32)

        # ======================= LSTM phase =======================
        with tc.tile_pool(name="wx", bufs=3, space="PSUM") as wxp, \
             tc.tile_pool(name="psA", bufs=1, space="PSUM") as psA, \
             tc.tile_pool(name="psE", bufs=2, space="PSUM") as psE, \
             tc.tile_pool(name="psT", bufs=2, space="PSUM") as psT, \
             tc.tile_pool(name="gath", bufs=3) as gath, \
             tc.tile_pool(name="sgp", bufs=3) as sgp, \
             tc.tile_pool(name="mp", bufs=2) as mp, \
             tc.tile_pool(name="gw", bufs=3) as gw:

            # gather order: ends-first pairs (bwd needs high t, fwd low t)
            gather_order = []
            for i in range((M + 1) // 2):
                gather_order.append(M - 1 - i)
                if i != M - 1 - i:
                    gather_order.append(i)

            def emit_gather(g):
                m = gather_order[g]
                xe_raw = gath.tile([128, 128], F32, tag="xe_raw")
                nc.gpsimd.indirect_dma_start(
                    out=xe_raw[:], out_offset=None,
                    in_=ins["emb"][:],
                    in_offset=IndirectOffsetOnAxis(ap=idx_sb[:, m:m + 1],
                                                   axis=0),
                )
                xe_ps = psA.tile([128, 128], F32, tag="xe_ps")
                nc.tensor.transpose(xe_ps[:], xe_raw[:], ident[:])
                nc.vector.tensor_copy(xeT[:, m * 128:(m + 1) * 128], xe_ps[:])

            wx_tiles = {}

            def emit_precompute(ci):
                wx = wxp.tile([128, 2, 4, CH, Bl], F32, tag="wx")
                wx_tiles[ci] = wx
                t0 = ci * CH
                first = True
                for k in range(4):  # fwd: one matmul per gate, N=CH*Bl
                    nc.tensor.matmul(
                        wx[:, 0, k, :, :],
                        lhsT=wih_sb[:, 0, k * 128:(k + 1) * 128],
                        rhs=xeT[:, t0 * Bl:(t0 + CH) * Bl],
                        start=first, stop=False)
                    first = False
                for k in range(4):  # bwd: per (gate, ri), N=Bl
                    for ri in range(CH):
                        tb = T - 1 - (t0 + ri)
                        nc.tensor.matmul(
                            wx[:, 1, k, ri, :],
                            lhsT=wih_sb[:, 1, k * 128:(k + 1) * 128],
                            rhs=xeT[:, tb * Bl:(tb + 1) * Bl],
                            start=False, stop=False)
                for dk in range(8):  # bias add, K=1
                    nc.tensor.matmul(
                        wx[:, dk // 4, dk % 4, :, :],
                        lhsT=bias8_sb[0:1, dk * 128:(dk + 1) * 128],
                        rhs=onesrow[0:1, :],
                        start=False, stop=False)

            # ---- slack-work emitters ----
            n_ch8 = T // TG     # 64 eight-position chunks

            def emit_emission(k):
                c0, c1 = k * TG * Bl, (k + 1) * TG * Bl
                ps = psE.tile([NT, TG * Bl], F32, tag="em_ps")
                nc.tensor.matmul(ps[:], lhsT=wout_sb[:, 0, :],
                                 rhs=h_f[:, c0:c1], start=True, stop=False)
                nc.tensor.matmul(ps[:], lhsT=wout_sb[:, 1, :],
                                 rhs=h_b[:, c0:c1], start=False, stop=True)
                nc.vector.tensor_scalar_add(emT[:, c0:c1], ps[:], bout_sb[:])

            def emit_gold_em(k):
                c0, c1 = k * TG * Bl, (k + 1) * TG * Bl
                mm1 = gw.tile([NT, TG * Bl], F32, tag="mm1")
                nc.vector.tensor_mul(mm1[:], w1_sb[:, c0:c1], emT[:, c0:c1])
                red = gw.tile([NT, Bl], F32, tag="red")
                nc.vector.tensor_reduce(
                    red[:], mm1[:].rearrange("p (t b) -> p b t", b=Bl),
                    axis=AX.X, op=OP.add)
                nc.vector.tensor_add(accE[:], accE[:], red[:])

            def emit_gold_trans(j):
                nt_ = min(TG, T - 1 - j * TG)
                c0 = j * TG * Bl
                trp = psT.tile([NT, TG * Bl], F32, tag="trp")
                nc.tensor.matmul(trp[:, 0:nt_ * Bl], lhsT=trans_sb[:],
                                 rhs=s1_sb[:, c0:c0 + nt_ * Bl],
                                 start=True, stop=True)
                mm2 = gw.tile([NT, TG * Bl], F32, tag="mm2")
                nc.vector.tensor_mul(mm2[:, 0:nt_ * Bl],
                                     s2_sb[:, c0:c0 + nt_ * Bl],
                                     trp[:, 0:nt_ * Bl])
                red2 = gw.tile([NT, Bl], F32, tag="red2")
                nc.vector.tensor_reduce(
                    red2[:, 0:Bl],
                    mm2[:, 0:nt_ * Bl].rearrange("p (t b) -> p b t", b=Bl),
                    axis=AX.X, op=OP.add)
                nc.vector.tensor_add(accT[:], accT[:], red2[:])

            # lead-in: first gathers + chunk 0 Wx
            for g in range(8):
                emit_gather(g)
            next_gather = 8
            emit_precompute(0)

            # ---------------- the coupled recurrence ----------------
            for r in range(T):
                ci, ri = divmod(r, CH)
                wx = wx_tiles[ci]
                if ri == 0 and ci + 1 < NCH:
                    emit_precompute(ci + 1)
                    wx_tiles.pop(ci - 2, None)
                if r > 0:
                    for d in (0, 1):
                        rhs = (h_f[:, (r - 1) * Bl:r * Bl] if d == 0
                               else h_b[:, (T - r) * Bl:(T - r + 1) * Bl])
                        for k in range(4):
                            last = (ri == CH - 1) and d == 1 and k == 3
                            nc.tensor.matmul(
                                wx[:, d, k, ri, :],
                                lhsT=whh_sb[:, d, k * 128:(k + 1) * 128],
                                rhs=rhs, start=False, stop=last)
                sg = sgp.tile([128, 2, 4, Bl], F32, tag="sg")
                nc.scalar.activation(sg[:], wx[:, :, :, ri, :], ACTF.Sigmoid)
                m1 = mp.tile([128, 2, Bl], F32, tag="m1")
                nc.vector.scalar_tensor_tensor(
                    m1[:], sg[:, :, 3, :], 0.5, sg[:, :, 0, :],
                    op0=OP.subtract, op1=OP.mult)
                m2 = mp.tile([128, 2, Bl], F32, tag="m2")
                nc.vector.tensor_mul(m2[:], sg[:, :, 1, :], cst[:])
                nc.vector.scalar_tensor_tensor(
                    cst[:], m1[:], 2.0, m2[:], op0=OP.mult, op1=OP.add)
                tcn = mp.tile([128, 2, Bl], F32, tag="tcn")
                nc.scalar.activation(tcn[:], cst[:], ACTF.Sigmoid, scale=2.0)
                nc.vector.scalar_tensor_tensor(
                    h_f[:, r * Bl:(r + 1) * Bl], tcn[:, 0, :], 0.5,
                    sg[:, 0, 2, :], op0=OP.subtract, op1=OP.mult)
                nc.vector.scalar_tensor_tensor(
                    h_b[:, (T - 1 - r) * Bl:(T - r) * Bl], tcn[:, 1, :], 0.5,
                    sg[:, 1, 2, :], op0=OP.subtract, op1=OP.mult)

                # ---- gathers feed the chain; keep them paced in-loop ----
                if next_gather < M and r % 3 == 1:
                    emit_gather(next_gather)
                    next_gather += 1

            # emissions + gold: emitted after the loop in readiness order;
            # the list scheduler backfills them into engine-idle windows.
            for k in sorted(range(n_ch8),
                            key=lambda k: max(TG * k + TG - 1, T - 1 - TG * k)):
                emit_emission(k)
                emit_gold_em(k)
            for j in range(n_ch8):
                emit_gold_trans(j)

        # ======================= CRF phase =======================
        with tc.tile_pool(name="psCa", bufs=2, space="PSUM") as psCa, \
             tc.tile_pool(name="psCg", bufs=2, space="PSUM") as psCg, \
             tc.tile_pool(name="psOB", bufs=2, space="PSUM") as psOB, \
             tc.tile_pool(name="psD", bufs=1, space="PSUM") as psD, \
             tc.tile_pool(name="cw", bufs=2) as cw, \
             tc.tile_pool(name="cw2", bufs=2) as cw2:

            # token written only after the last LSTM round: gates every Exp
            # op so the scheduler cannot hoist them into the Sigmoid loop
            # (each hoist would cost 2x1283ns act-table loads on the chain).
            tok = cw.tile([NT, 1], F32, tag="tok")
            nc.vector.tensor_scalar_mul(tok[:], h_b[0:NT, 0:1], 0.0)
            # exp(emissions), ends-first so both chains start immediately
            exp_order = []
            for i in range(n_ch8 // 2):
                exp_order.extend([n_ch8 - 1 - i, i])
            for k in exp_order:
                c0, c1 = k * TG * Bl, (k + 1) * TG * Bl
                nc.scalar.activation(EM[:, c0:c1], emT[:, c0:c1], ACTF.Exp,
                                     bias=tok[:])

            # alpha init: q0 = exp(start) * EM[:, 0]
            nc.vector.tensor_scalar_mul(qa[:], EM[:, 0:Bl], estart_sb[:])
            # gamma seed: gamma_{T-1} = exp(end) (x) sel{len==T}
            Pg = psCg.tile([NT, Bl], F32, tag="pg")
            nc.tensor.matmul(Pg[:], lhsT=einj_sb[:], rhs=injsel_sb[:, 0:Bl],
                             start=True, stop=True)

            inj_at = {T - 1 - te: e for e, te in enumerate(IEV)}
            emp_a = {}
            emp_g = {}

            def rebase(tag, src_row, hist, slot, em_cols, store):
                """Off-chain rebase: snapshot src_row -> clamp to hist slot,
                reciprocal, broadcast, scale EM[:, em_cols] into store dict."""
                hs = hist[0:1, slot * Bl:(slot + 1) * Bl]
                nc.vector.tensor_scalar_max(hs, src_row, 1.0)
                rc = cw.tile([1, Bl], F32, tag=f"rc{tag}")
                nc.vector.reciprocal(rc[:], hs)
                ob = psOB.tile([NT, Bl], F32, tag="ob")
                nc.tensor.matmul(ob[:], lhsT=ones1[:], rhs=rc[:],
                                 start=True, stop=True)
                if em_cols is None:
                    return ob
                emp = cw2.tile([NT, Bl], F32, tag=f"emp{tag}")
                nc.vector.tensor_mul(emp[:], EM[:, em_cols[0]:em_cols[1]],
                                     ob[:])
                store[em_cols[2]] = emp
                return None

            for s in range(T // 2):
                # gamma step: processes position t, produces gamma_{t-1}
                t = T - 1 - s
                gtmp = cw.tile([NT, Bl], F32, tag="gtmp")
                em_g = emp_g.pop(s, None)
                em_ap = em_g[:] if em_g is not None \
                    else EM[:, t * Bl:(t + 1) * Bl]
                nc.vector.tensor_mul(gtmp[:], em_ap, Pg[:])
                e = inj_at.get(s)
                Pg = psCg.tile([NT, Bl], F32, tag="pg")
                nc.tensor.matmul(Pg[:], lhsT=EtrT_sb[:], rhs=gtmp[:],
                                 start=True, stop=(e is None))
                if e is not None:
                    nc.tensor.matmul(
                        Pg[:], lhsT=einj_sb[:],
                        rhs=injsel_sb[:, (e + 1) * Bl:(e + 2) * Bl],
                        start=False, stop=True)
                # alpha step: position ta
                if s < T // 2 - 1:
                    ta = s + 1
                    Pa = psCa.tile([NT, Bl], F32, tag="pa")
                    nc.tensor.matmul(Pa[:], lhsT=Etr_sb[:], rhs=qa[:],
                                     start=True, stop=True)
                    em_a = emp_a.pop(ta, None)
                    ema_ap = em_a[:] if em_a is not None \
                        else EM[:, ta * Bl:(ta + 1) * Bl]
                    nc.vector.tensor_mul(qa[:], Pa[:], ema_ap)
                    if ta % RB == 0 and ta <= 252:
                        snap = cw.tile([1, Bl], F32, tag="snapa")
                        nc.vector.tensor_mul(snap[:], Pa[0:1, :],
                                             ema_ap[0:1, :])
                        tap = ta + LAG
                        rebase("a", snap[:], hista, ta // RB - 1,
                               (tap * Bl, (tap + 1) * Bl, tap), emp_a)
                if s % RB == 1 and 5 <= s <= 253:
                    tgp = T - 1 - (s + LAG)
                    rebase("g", Pg[0:1, :], histg, (s - 5) // RB,
                           (tgp * Bl, (tgp + 1) * Bl, s + LAG), emp_g)

            # terminal rebases + combine
            oba = rebase("a", qa[0:1, :], hista, NSH - 1, None, None)
            nc.vector.tensor_mul(qa[:], qa[:], oba[:])
            obg = rebase("g", Pg[0:1, :], histg, NSH - 1, None, None)
            nc.vector.tensor_mul(qa[:], qa[:], obg[:])
            de = cw.tile([NT, Bl], F32, tag="de")
            nc.vector.tensor_mul(de[:], qa[:], Pg[:])
            dsum = psD.tile([1, Bl], F32, tag="dsum")
            nc.tensor.matmul(dsum[:], lhsT=ones24[:], rhs=de[:],
                             start=True, stop=True)

            # gold total
            nc.vector.tensor_add(accE[:], accE[:], accT[:])
            gsum = psD.tile([1, Bl], F32, tag="gsum")
            nc.tensor.matmul(gsum[:], lhsT=ones24[:], rhs=accE[:],
                             start=True, stop=False)
            nc.tensor.matmul(gsum[:], lhsT=startv[:], rhs=selstart_sb[:],
                             start=False, stop=False)
            nc.tensor.matmul(gsum[:], lhsT=endv[:], rhs=selend_sb[:],
                             start=False, stop=True)

            # offsets: A = sum_s ln(hist)
            lnA = cw2.tile([1, NSH * Bl], F32, tag="lnA")
            nc.scalar.activation(lnA[:], hista[:], ACTF.Ln)
            Aa = cw.tile([1, Bl], F32, tag="Aa")
            nc.vector.tensor_reduce(
                Aa[:], lnA[:].rearrange("p (s b) -> p b s", b=Bl),
                axis=AX.X, op=OP.add)
            lnG = cw2.tile([1, NSH * Bl], F32, tag="lnG")
            nc.scalar.activation(lnG[:], histg[:], ACTF.Ln)
            Ag = cw.tile([1, Bl], F32, tag="Ag")
            nc.vector.tensor_reduce(
                Ag[:], lnG[:].rearrange("p (s b) -> p b s", b=Bl),
                axis=AX.X, op=OP.add)
            logd = cw.tile([1, Bl], F32, tag="logd")
            nc.scalar.activation(logd[:], dsum[:], ACTF.Ln)

            lr = cw.tile([1, Bl], F32, tag="lr")
            nc.vector.tensor_add(lr[:], logd[:], Aa[:])
            nc.vector.tensor_add(lr[:], lr[:], Ag[:])
            nc.vector.tensor_tensor(lr[:], lr[:], gsum[:], op=OP.subtract)
            nc.sync.dma_start(out=outs["loss"][:].unsqueeze(0), in_=lr[:])


# ======================= host-side preparation =======================

def make_core_inputs(cfg, x, tags, mask, emb, Wih_f, Whh_f, bih_f, bhh_f,
                     Wih_b, Whh_b, bih_b, bhh_b, W_out, b_out,
                     transitions, start_trans, end_trans):
    """Per-core input map (numpy). x/tags/mask are the LOCAL [Bl, T] slices."""
    T, Bl, NT, Hd = cfg["T"], cfg["Bl"], cfg["NT"], cfg["Hd"]
    R = T * Bl
    M = R // 128
    perm = [0, 1, 3, 2]  # torch gate order (i,f,g,o) -> ours (i,f,o,g)

    WIH_S = np.array([1.0, 1.0, 1.0, 2.0], np.float32)   # (i,f,o,g)
    WHH_S = np.array([2.0, 2.0, 2.0, 4.0], np.float32)

    def reorder_rows(w, scales):
        blocks = [w[k * Hd:(k + 1) * Hd] * s for k, s in zip(perm, scales)]
        return np.concatenate(blocks, axis=0)

    def pack_w(wf, wb, scales):
        out = np.empty((128, 2, 4 * Hd), dtype=ml_dtypes.bfloat16)
        out[:, 0, :] = reorder_rows(np.asarray(wf, np.float32), scales).T
        out[:, 1, :] = reorder_rows(np.asarray(wb, np.float32), scales).T
        return out

    def pack_bias(bi, bh):  # -> [4, 128] rows = gates (i,f,o,g)
        b = np.asarray(bi, np.float32) + np.asarray(bh, np.float32)
        return reorder_rows(b, WIH_S).reshape(4, Hd)

    bias8 = np.empty((8, 128), np.float32)
    bias8[0:4] = pack_bias(bih_f, bhh_f)
    bias8[4:8] = pack_bias(bih_b, bhh_b)
    bias8 = bias8.reshape(1, 8 * 128)

    W_out = np.asarray(W_out, np.float32) * 2.0   # h stored as h/2
    wout = np.empty((128, 2, NT), dtype=ml_dtypes.bfloat16)
    wout[:, 0, :] = W_out[:, :Hd].T
    wout[:, 1, :] = W_out[:, Hd:].T

    x = np.asarray(x)
    tags = np.asarray(tags)
    maskf = np.asarray(mask).astype(np.float32)
    trans = np.ascontiguousarray(transitions, np.float32)
    transb = trans.astype(ml_dtypes.bfloat16)
    start_trans = np.asarray(start_trans, np.float32)
    end_trans = np.asarray(end_trans, np.float32)

    x_tm = x.T.reshape(-1).astype(np.int32)
    idx = x_tm.reshape(M, 128).T.copy()

    eye = np.eye(NT, dtype=np.float32)
    w = maskf.copy()
    w[:, 0] = 1.0
    w1 = eye[tags]
    w1 = (w1 * w[:, :, None]).transpose(2, 1, 0)
    w1hot = np.ascontiguousarray(w1.reshape(NT, R), np.float32)

    sel1 = eye[tags[:, :-1]].transpose(2, 1, 0).reshape(NT, (T - 1) * Bl)
    sel1 = np.ascontiguousarray(sel1, np.float32)
    sel2 = eye[tags[:, 1:]] * maskf[:, 1:, None]
    sel2m = np.ascontiguousarray(
        sel2.transpose(2, 1, 0).reshape(NT, (T - 1) * Bl), np.float32)

    selstart = np.ascontiguousarray(eye[tags[:, 0]].T, np.float32)
    lens = np.asarray(mask).sum(axis=1).astype(np.int64)
    last_tags = tags[np.arange(Bl), lens - 1]
    selend = np.ascontiguousarray(eye[last_tags].T, np.float32)

    IEV = cfg["IEV"]
    injsel = np.zeros((1, (len(IEV) + 1) * Bl), np.float32)
    injsel[0, 0:Bl] = (lens == T).astype(np.float32)
    for e, te in enumerate(IEV):
        injsel[0, (e + 1) * Bl:(e + 2) * Bl] = \
            (lens == te).astype(np.float32)

    return {
        "emb": np.ascontiguousarray(emb, np.float32),
        "idx": idx,
        "wih": pack_w(Wih_f, Wih_b, WIH_S),
        "whh": pack_w(Whh_f, Whh_b, WHH_S),
        "wout": wout,
        "bias8": bias8,
        "bout": np.asarray(b_out, np.float32).reshape(NT, 1),
        "trans": transb,
        "Etrans": np.exp(trans).astype(np.float32),
        "EtransT": np.ascontiguousarray(np.exp(trans).T, np.float32),
        "estart": np.exp(start_trans).astype(np.float32).reshape(NT, 1),
        "einj": np.exp(end_trans).astype(np.float32).reshape(1, NT),
        "injsel": injsel,
        "selstart": selstart,
        "selend": selend,
        "startv": start_trans.reshape(NT, 1).astype(np.float32),
        "endv": end_trans.reshape(NT, 1).astype(np.float32),
        "w1hot": w1hot.astype(ml_dtypes.bfloat16),
        "sel1": sel1.astype(ml_dtypes.bfloat16),
        "sel2m": sel2m.astype(ml_dtypes.bfloat16),
    }


def input_specs(cfg):
    T, Bl, NT, Hd, V = cfg["T"], cfg["Bl"], cfg["NT"], cfg["Hd"], cfg["V"]
    R = T * Bl
    M = R // 128
    NE = len(cfg["IEV"])
    return {
        "emb": ([V, 128], F32),
        "idx": ([128, M], I32),
        "wih": ([128, 2, 4 * Hd], BF16),
        "whh": ([128, 2, 4 * Hd], BF16),
        "wout": ([128, 2, NT], BF16),
        "bias8": ([1, 8 * 128], F32),
        "bout": ([NT, 1], F32),
        "trans": ([NT, NT], BF16),
        "Etrans": ([NT, NT], F32),
        "EtransT": ([NT, NT], F32),
        "estart": ([NT, 1], F32),
        "einj": ([1, NT], F32),
        "injsel": ([1, (NE + 1) * Bl], F32),
        "selstart": ([NT, Bl], F32),
        "selend": ([NT, Bl], F32),
        "startv": ([NT, 1], F32),
        "endv": ([NT, 1], F32),
        "w1hot": ([NT, R], BF16),
        "sel1": ([NT, (T - 1) * Bl], BF16),
        "sel2m": ([NT, (T - 1) * Bl], BF16),
    }


_BUILT = {}


def build_program(cfg, num_devices=8):
    key = tuple((k, v) for k, v in sorted(cfg.items()))
    if key in _BUILT:
        return _BUILT[key]
    nc = bacc.Bacc("TRN2", target_bir_lowering=False, debug=False,
                   num_devices=num_devices)
    ins = {}
    for name, (shape, dt_) in input_specs(cfg).items():
        ins[name] = nc.dram_tensor(name, shape, dt_, kind="ExternalInput").ap()
    outs = {"loss": nc.dram_tensor("loss", [cfg["Bl"]], F32,
                                   kind="ExternalOutput").ap()}
    with tile.TileContext(nc) as tc:
        build_body(tc, outs, ins, cfg)
    nc.compile()
    _BUILT[key] = nc
    return nc


def kernel(**inputs):
    from concourse.bass_utils import run_bass_kernel_spmd

    cfg = full_cfg()
    Bl = cfg["Bl"]
    B = 128
    T = cfg["T"]
    n_cores = B // Bl

    np_in = {k: np.asarray(v) for k, v in inputs.items()}
    lens = np_in["mask"].sum(axis=1).astype(np.int64)
    iev = tuple(sorted({int(l) for l in lens if l < T}))
    cfg = dict(cfg, IEV=iev)
    nc = build_program(cfg, num_devices=n_cores)
    in_maps = []
    for c in range(n_cores):
        sl = slice(c * Bl, (c + 1) * Bl)
        in_maps.append(make_core_inputs(
            cfg,
            np_in["x"][sl], np_in["tags"][sl], np_in["mask"][sl],
            np_in["emb"],
            np_in["Wih_f"], np_in["Whh_f"], np_in["bih_f"], np_in["bhh_f"],
            np_in["Wih_b"], np_in["Whh_b"], np_in["bih_b"], np_in["bhh_b"],
            np_in["W_out"], np_in["b_out"], np_in["transitions"],
            np_in["start_trans"], np_in["end_trans"]))

    res = run_bass_kernel_spmd(nc, in_maps, core_ids=list(range(n_cores)),
                               trace=TRACE)
    if res.exec_time_ns is not None:
        LAST_EXEC_NS.append(res.exec_time_ns)
    vals = np.concatenate([res.results[c]["loss"] for c in range(n_cores)])
    return np.float32(vals.mean())


TRACE = False
LAST_EXEC_NS = []


# revision 12
# speedup vs baseline: 1.2089x; 1.0124x over previous
"""BiLSTM-CRF loss kernel for Trainium2 (8 NeuronCores, data-parallel over batch).

Self-contained: hardcodes shapes B=128, T=512, V=50000, NT=24, E=128, H=256.
Each core processes 16 examples end-to-end.

v2 design (latency-oriented; the kernel is serial-chain bound, not
throughput bound):
  * LSTM: ONE coupled chain per round computing both directions' step
    (one [128,2,4,16] sigmoid, shared cell ops via strided APs).  The
    input projections Wih@x+bias are precomputed chunk-wise directly
    into the PSUM banks that the in-loop Whh@h matmuls then accumulate
    onto (PSUM "pending zero" semantics make this exact).
  * Emissions (W_out@h+b, no exp) and the gold-path reductions are
    interleaved into the LSTM rounds' engine-idle windows.
  * exp() of emissions runs post-LSTM (avoids Act table thrash).
  * CRF: exp-domain, split meet-in-the-middle: alpha recursion over
    positions 0..255 (always unmasked since lengths >= 256) runs
    CONCURRENTLY with the backward (beta/gamma) recursion over
    positions 256..511; per-example end/masking enters gamma via
    exp(end) injections at data-dependent steps.  logZ = ln(sum_i
    alpha_i*gamma_i) + rebase offsets.  Rebasing (overflow control) is
    done OFF the critical chain: scale factors fold into a later
    step's emission operand (legal because the recursion is linear).
"""

import sys

for _p in ("/opt/trn_rl_repo",):
    if _p not in sys.path:
        sys.path.insert(0, _p)

import numpy as np
import ml_dtypes

import concourse.bass as bass
import concourse.bacc as bacc
import concourse.tile as tile
from concourse import mybir
from concourse.bass import IndirectOffsetOnAxis
from concourse.masks import make_identity

F32 = mybir.dt.float32
BF16 = mybir.dt.bfloat16
I32 = mybir.dt.int32
AX = mybir.AxisListType
OP = mybir.AluOpType
ACTF = mybir.ActivationFunctionType


def full_cfg():
    return dict(T=512, Bl=16, V=50000, NT=24, E=128, Hd=128,
                CH=4, RB=8, LAG=2, TG=8)


def build_body(tc, outs, ins, cfg):
    nc = tc.nc
    T, Bl, NT, Hd = cfg["T"], cfg["Bl"], cfg["NT"], cfg["Hd"]
    CH, RB, LAG, TG = cfg["CH"], cfg["RB"], cfg["LAG"], cfg["TG"]
    R = T * Bl
    M = R // 128            # gather tiles (8 positions each)
    NCH = T // CH           # Wx chunks
    IEV = cfg["IEV"]        # sorted distinct lengths in [256, 511]
    NSH = 32                # rebase history slots per chain

    import contextlib
    ctx = contextlib.ExitStack()
    with ctx:
        const = ctx.enter_context(tc.tile_pool(name="const", bufs=1))
        big = ctx.enter_context(tc.tile_pool(name="big", bufs=1))

        # ---------------- constants ----------------
        ident = const.tile([128, 128], F32)
        make_identity(nc, ident[:])

        idx_sb = const.tile([128, M], I32)
        nc.sync.dma_start(out=idx_sb[:], in_=ins["idx"][:])
        wih_sb = const.tile([128, 2, 4 * Hd], BF16)
        nc.sync.dma_start(out=wih_sb[:], in_=ins["wih"][:])
        whh_sb = const.tile([128, 2, 4 * Hd], BF16)
        nc.sync.dma_start(out=whh_sb[:], in_=ins["whh"][:])
        wout_sb = const.tile([128, 2, NT], BF16)
        nc.sync.dma_start(out=wout_sb[:], in_=ins["wout"][:])
        bias8_sb = const.tile([1, 8 * 128], F32)
        nc.sync.dma_start(out=bias8_sb[:], in_=ins["bias8"][:])
        bout_sb = const.tile([NT, 1], F32)
        nc.sync.dma_start(out=bout_sb[:], in_=ins["bout"][:])
        trans_sb = const.tile([NT, NT], BF16)
        nc.sync.dma_start(out=trans_sb[:], in_=ins["trans"][:])
        Etr_sb = const.tile([NT, NT], F32)
        nc.sync.dma_start(out=Etr_sb[:], in_=ins["Etrans"][:])
        EtrT_sb = const.tile([NT, NT], F32)
        nc.sync.dma_start(out=EtrT_sb[:], in_=ins["EtransT"][:])
        estart_sb = const.tile([NT, 1], F32)
        nc.sync.dma_start(out=estart_sb[:], in_=ins["estart"][:])
        einj_sb = const.tile([1, NT], F32)
        nc.sync.dma_start(out=einj_sb[:], in_=ins["einj"][:])
        injsel_sb = const.tile([1, (len(IEV) + 1) * Bl], F32)
        nc.sync.dma_start(out=injsel_sb[:], in_=ins["injsel"][:])
        selstart_sb = const.tile([NT, Bl], F32)
        nc.sync.dma_start(out=selstart_sb[:], in_=ins["selstart"][:])
        selend_sb = const.tile([NT, Bl], F32)
        nc.sync.dma_start(out=selend_sb[:], in_=ins["selend"][:])
        startv = const.tile([NT, 1], F32)
        nc.sync.dma_start(out=startv[:], in_=ins["startv"][:])
        endv = const.tile([NT, 1], F32)
        nc.sync.dma_start(out=endv[:], in_=ins["endv"][:])

        onesrow = const.tile([1, CH * Bl], F32)
        nc.vector.memset(onesrow[:], 1.0)
        ones1 = const.tile([1, NT], F32)
        nc.vector.memset(ones1[:], 1.0)
        ones24 = const.tile([NT, 1], F32)
        nc.vector.memset(ones24[:], 1.0)

        # ---------------- big persistent tensors ----------------
        xeT = big.tile([128, R], BF16)
        h_f = big.tile([128, R], BF16)
        h_b = big.tile([128, R], BF16)
        emT = big.tile([NT, R], F32)
        EM = big.tile([NT, R], F32)
        cst = big.tile([128, 2, Bl], F32)
        nc.vector.memset(cst[:], 0.0)
        accE = big.tile([NT, Bl], F32)
        nc.vector.memset(accE[:], 0.0)
        accT = big.tile([NT, Bl], F32)
        nc.vector.memset(accT[:], 0.0)
        w1_sb = big.tile([NT, R], BF16)
        nc.sync.dma_start(out=w1_sb[:], in_=ins["w1hot"][:])
        s1_sb = big.tile([NT, (T - 1) * Bl], BF16)
        nc.sync.dma_start(out=s1_sb[:], in_=ins["sel1"][:])
        s2_sb = big.tile([NT, (T - 1) * Bl], BF16)
        nc.sync.dma_start(out=s2_sb[:], in_=ins["sel2m"][:])
        hista = big.tile([1, NSH * Bl], F32)
        nc.vector.memset(hista[:], 1.0)
        histg = big.tile([1, NSH * Bl], F32)
        nc.vector.memset(histg[:], 1.0)
        qa = big.tile([NT, Bl], F32)

        # ======================= LSTM phase =======================
        with tc.tile_pool(name="wx", bufs=3, space="PSUM") as wxp, \
             tc.tile_pool(name="psA", bufs=1, space="PSUM") as psA, \
             tc.tile_pool(name="psE", bufs=2, space="PSUM") as psE, \
             tc.tile_pool(name="psT", bufs=2, space="PSUM") as psT, \
             tc.tile_pool(name="gath", bufs=3) as gath, \
             tc.tile_pool(name="sgp", bufs=3) as sgp, \
             tc.tile_pool(name="mp", bufs=2) as mp, \
             tc.tile_pool(name="gw", bufs=3) as gw:

            # gather order: ends-first pairs (bwd needs high t, fwd low t)
            gather_order = []
            for i in range((M + 1) // 2):
                gather_order.append(M - 1 - i)
                if i != M - 1 - i:
                    gather_order.append(i)

            def emit_gather(g):
                m = gather_order[g]
                xe_raw = gath.tile([128, 128], F32, tag="xe_raw")
                nc.gpsimd.indirect_dma_start(
                    out=xe_raw[:], out_offset=None,
                    in_=ins["emb"][:],
                    in_offset=IndirectOffsetOnAxis(ap=idx_sb[:, m:m + 1],
                                                   axis=0),
                )
                xe_ps = psA.tile([128, 128], F32, tag="xe_ps")
                nc.tensor.transpose(xe_ps[:], xe_raw[:], ident[:])
                nc.vector.tensor_copy(xeT[:, m * 128:(m + 1) * 128], xe_ps[:])

            wx_tiles = {}

            def emit_precompute(ci):
                wx = wxp.tile([128, 2, 4, CH, Bl], F32, tag="wx")
                wx_tiles[ci] = wx
                t0 = ci * CH
                first = True
                for k in range(4):  # fwd: one matmul per gate, N=CH*Bl
                    nc.tensor.matmul(
                        wx[:, 0, k, :, :],
                        lhsT=wih_sb[:, 0, k * 128:(k + 1) * 128],
                        rhs=xeT[:, t0 * Bl:(t0 + CH) * Bl],
                        start=first, stop=False)
                    first = False
                for k in range(4):  # bwd: per (gate, ri), N=Bl
                    for ri in range(CH):
                        tb = T - 1 - (t0 + ri)
                        nc.tensor.matmul(
                            wx[:, 1, k, ri, :],
                            lhsT=wih_sb[:, 1, k * 128:(k + 1) * 128],
                            rhs=xeT[:, tb * Bl:(tb + 1) * Bl],
                            start=False, stop=False)
                for dk in range(8):  # bias add, K=1
                    nc.tensor.matmul(
                        wx[:, dk // 4, dk % 4, :, :],
                        lhsT=bias8_sb[0:1, dk * 128:(dk + 1) * 128],
                        rhs=onesrow[0:1, :],
                        start=False, stop=False)

            # ---- slack-work emitters ----
            n_ch8 = T // TG     # 64 eight-position chunks

            def emit_emission(k):
                c0, c1 = k * TG * Bl, (k + 1) * TG * Bl
                ps = psE.tile([NT, TG * Bl], F32, tag="em_ps")
                nc.tensor.matmul(ps[:], lhsT=wout_sb[:, 0, :],
                                 rhs=h_f[:, c0:c1], start=True, stop=False)
                nc.tensor.matmul(ps[:], lhsT=wout_sb[:, 1, :],
                                 rhs=h_b[:, c0:c1], start=False, stop=True)
                nc.vector.tensor_scalar_add(emT[:, c0:c1], ps[:], bout_sb[:])

            def emit_gold_em(k):
                c0, c1 = k * TG * Bl, (k + 1) * TG * Bl
                mm1 = gw.tile([NT, TG * Bl], F32, tag="mm1")
                nc.vector.tensor_mul(mm1[:], w1_sb[:, c0:c1], emT[:, c0:c1])
                red = gw.tile([NT, Bl], F32, tag="red")
                nc.vector.tensor_reduce(
                    red[:], mm1[:].rearrange("p (t b) -> p b t", b=Bl),
                    axis=AX.X, op=OP.add)
                nc.vector.tensor_add(accE[:], accE[:], red[:])

            def emit_gold_trans(j):
                nt_ = min(TG, T - 1 - j * TG)
                c0 = j * TG * Bl
                trp = psT.tile([NT, TG * Bl], F32, tag="trp")
                nc.tensor.matmul(trp[:, 0:nt_ * Bl], lhsT=trans_sb[:],
                                 rhs=s1_sb[:, c0:c0 + nt_ * Bl],
                                 start=True, stop=True)
                mm2 = gw.tile([NT, TG * Bl], F32, tag="mm2")
                nc.vector.tensor_mul(mm2[:, 0:nt_ * Bl],
                                     s2_sb[:, c0:c0 + nt_ * Bl],
                                     trp[:, 0:nt_ * Bl])
                red2 = gw.tile([NT, Bl], F32, tag="red2")
                nc.vector.tensor_reduce(
                    red2[:, 0:Bl],
                    mm2[:, 0:nt_ * Bl].rearrange("p (t b) -> p b t", b=Bl),
                    axis=AX.X, op=OP.add)
                nc.vector.tensor_add(accT[:], accT[:], red2[:])

            # lead-in: first gathers + chunk 0 Wx
            for g in range(8):
                emit_gather(g)
            next_gather = 8
            emit_precompute(0)

            # ---------------- the coupled recurrence ----------------
            for r in range(T):
                ci, ri = divmod(r, CH)
                wx = wx_tiles[ci]
                if ri == 0 and ci + 1 < NCH:
                    emit_precompute(ci + 1)
                    wx_tiles.pop(ci - 2, None)
                if r > 0:
                    for d in (0, 1):
                        rhs = (h_f[:, (r - 1) * Bl:r * Bl] if d == 0
                               else h_b[:, (T - r) * Bl:(T - r + 1) * Bl])
                        for k in range(4):
                            last = (ri == CH - 1) and d == 1 and k == 3
                            nc.tensor.matmul(
                                wx[:, d, k, ri, :],
                                lhsT=whh_sb[:, d, k * 128:(k + 1) * 128],
                                rhs=rhs, start=False, stop=last)
                sg = sgp.tile([128, 2, 4, Bl], F32, tag="sg")
                nc.scalar.activation(sg[:], wx[:, :, :, ri, :], ACTF.Sigmoid)
                m1 = mp.tile([128, 2, Bl], F32, tag="m1")
                nc.vector.scalar_tensor_tensor(
                    m1[:], sg[:, :, 3, :], 0.5, sg[:, :, 0, :],
                    op0=OP.subtract, op1=OP.mult)
                m2 = mp.tile([128, 2, Bl], F32, tag="m2")
                nc.vector.tensor_mul(m2[:], sg[:, :, 1, :], cst[:])
                nc.vector.scalar_tensor_tensor(
                    cst[:], m1[:], 2.0, m2[:], op0=OP.mult, op1=OP.add)
                tcn = mp.tile([128, 2, Bl], F32, tag="tcn")
                nc.scalar.activation(tcn[:], cst[:], ACTF.Sigmoid, scale=2.0)
                nc.vector.scalar_tensor_tensor(
                    h_f[:, r * Bl:(r + 1) * Bl], tcn[:, 0, :], 0.5,
                    sg[:, 0, 2, :], op0=OP.subtract, op1=OP.mult)
                nc.vector.scalar_tensor_tensor(
                    h_b[:, (T - 1 - r) * Bl:(T - r) * Bl], tcn[:, 1, :], 0.5,
                    sg[:, 1, 2, :], op0=OP.subtract, op1=OP.mult)

                # ---- gathers feed the chain; keep them paced in-loop ----
                if next_gather < M and r % 3 == 1:
                    emit_gather(next_gather)
                    next_gather += 1

            # emissions + gold: emitted after the loop in readiness order;
            # the list scheduler backfills them into engine-idle windows.
            for k in sorted(range(n_ch8),
                            key=lambda k: max(TG * k + TG - 1, T - 1 - TG * k)):
                emit_emission(k)
                emit_gold_em(k)
            for j in range(n_ch8):
                emit_gold_trans(j)

        # ======================= CRF phase =======================
        with tc.tile_pool(name="psCa", bufs=2, space="PSUM") as psCa, \
             tc.tile_pool(name="psCg", bufs=2, space="PSUM") as psCg, \
             tc.tile_pool(name="psOB", bufs=2, space="PSUM") as psOB, \
             tc.tile_pool(name="psD", bufs=1, space="PSUM") as psD, \
             tc.tile_pool(name="cw", bufs=2) as cw, \
             tc.tile_pool(name="cw2", bufs=2) as cw2:

            # token written only after the last LSTM round: gates every Exp
            # op so the scheduler cannot hoist them into the Sigmoid loop
            # (each hoist would cost 2x1283ns act-table loads on the chain).
            tok = cw.tile([NT, 1], F32, tag="tok")
            nc.vector.tensor_scalar_mul(tok[:], h_b[0:NT, 0:1], 0.0)
            # exp(emissions), ends-first so both chains start immediately
            exp_order = []
            for i in range(n_ch8 // 2):
                exp_order.extend([n_ch8 - 1 - i, i])
            for k in exp_order:
                c0, c1 = k * TG * Bl, (k + 1) * TG * Bl
                nc.scalar.activation(EM[:, c0:c1], emT[:, c0:c1], ACTF.Exp,
                                     bias=tok[:])

            # alpha init: q0 = exp(start) * EM[:, 0]
            nc.vector.tensor_scalar_mul(qa[:], EM[:, 0:Bl], estart_sb[:])
            # gamma seed: gamma_{T-1} = exp(end) (x) sel{len==T}
            Pg = psCg.tile([NT, Bl], F32, tag="pg")
            nc.tensor.matmul(Pg[:], lhsT=einj_sb[:], rhs=injsel_sb[:, 0:Bl],
                             start=True, stop=True)

            inj_at = {T - 1 - te: e for e, te in enumerate(IEV)}
            emp_a = {}
            emp_g = {}

            def rebase(tag, src_row, hist, slot, em_cols, store):
                """Off-chain rebase: snapshot src_row -> clamp to hist slot,
                reciprocal, broadcast, scale EM[:, em_cols] into store dict."""
                hs = hist[0:1, slot * Bl:(slot + 1) * Bl]
                nc.vector.tensor_scalar_max(hs, src_row, 1.0)
                rc = cw.tile([1, Bl], F32, tag=f"rc{tag}")
                nc.vector.reciprocal(rc[:], hs)
                ob = psOB.tile([NT, Bl], F32, tag="ob")
                nc.tensor.matmul(ob[:], lhsT=ones1[:], rhs=rc[:],
                                 start=True, stop=True)
                if em_cols is None:
                    return ob
                emp = cw2.tile([NT, Bl], F32, tag=f"emp{tag}")
                nc.vector.tensor_mul(emp[:], EM[:, em_cols[0]:em_cols[1]],
                                     ob[:])
                store[em_cols[2]] = emp
                return None

            for s in range(T // 2):
                # gamma step: processes position t, produces gamma_{t-1}
                t = T - 1 - s
                gtmp = cw.tile([NT, Bl], F32, tag="gtmp")
                em_g = emp_g.pop(s, None)
                em_ap = em_g[:] if em_g is not None \
                    else EM[:, t * Bl:(t + 1) * Bl]
                nc.vector.tensor_mul(gtmp[:], em_ap, Pg[:])
                e = inj_at.get(s)
                Pg = psCg.tile([NT, Bl], F32, tag="pg")
                nc.tensor.matmul(Pg[:], lhsT=EtrT_sb[:], rhs=gtmp[:],
                                 start=True, stop=(e is None))
                if e is not None:
                    nc.tensor.matmul(
                        Pg[:], lhsT=einj_sb[:],
                        rhs=injsel_sb[:, (e + 1) * Bl:(e + 2) * Bl],
                        start=False, stop=True)
                # alpha step: position ta
                if s < T // 2 - 1:
                    ta = s + 1
                    Pa = psCa.tile([NT, Bl], F32, tag="pa")
                    nc.tensor.matmul(Pa[:], lhsT=Etr_sb[:], rhs=qa[:],
                                     start=True, stop=True)
                    em_a = emp_a.pop(ta, None)
                    ema_ap = em_a[:] if em_a is not None \
                        else EM[:, ta * Bl:(ta + 1) * Bl]
                    nc.vector.tensor_mul(qa[:], Pa[:], ema_ap)
                    if ta % RB == 0 and ta <= 253 - LAG:
                        snap = cw.tile([1, Bl], F32, tag="snapa")
                        nc.vector.tensor_mul(snap[:], Pa[0:1, :],
                                             ema_ap[0:1, :])
                        tap = ta + LAG
                        rebase("a", snap[:], hista, ta // RB - 1,
                               (tap * Bl, (tap + 1) * Bl, tap), emp_a)
                if s % RB == 5 and 5 <= s <= 253 - LAG:
                    tgp = T - 1 - (s + LAG)
                    rebase("g", Pg[0:1, :], histg, (s - 5) // RB,
                           (tgp * Bl, (tgp + 1) * Bl, s + LAG), emp_g)

            # terminal rebases + combine
            oba = rebase("a", qa[0:1, :], hista, NSH - 1, None, None)
            nc.vector.tensor_mul(qa[:], qa[:], oba[:])
            obg = rebase("g", Pg[0:1, :], histg, NSH - 1, None, None)
            nc.vector.tensor_mul(qa[:], qa[:], obg[:])
            de = cw.tile([NT, Bl], F32, tag="de")
            nc.vector.tensor_mul(de[:], qa[:], Pg[:])
            dsum = psD.tile([1, Bl], F32, tag="dsum")
            nc.tensor.matmul(dsum[:], lhsT=ones24[:], rhs=de[:],
                             start=True, stop=True)

            # gold total
            nc.vector.tensor_add(accE[:], accE[:], accT[:])
            gsum = psD.tile([1, Bl], F32, tag="gsum")
            nc.tensor.matmul(gsum[:], lhsT=ones24[:], rhs=accE[:],
                             start=True, stop=False)
            nc.tensor.matmul(gsum[:], lhsT=startv[:], rhs=selstart_sb[:],
                             start=False, stop=False)
            nc.tensor.matmul(gsum[:], lhsT=endv[:], rhs=selend_sb[:],
                             start=False, stop=True)

            # offsets: A = sum_s ln(hist)
            lnA = cw2.tile([1, NSH * Bl], F32, tag="lnA")
            nc.scalar.activation(lnA[:], hista[:], ACTF.Ln)
            Aa = cw.tile([1, Bl], F32, tag="Aa")
            nc.vector.tensor_reduce(
                Aa[:], lnA[:].rearrange("p (s b) -> p b s", b=Bl),
                axis=AX.X, op=OP.add)
            lnG = cw2.tile([1, NSH * Bl], F32, tag="lnG")
            nc.scalar.activation(lnG[:], histg[:], ACTF.Ln)
            Ag = cw.tile([1, Bl], F32, tag="Ag")
            nc.vector.tensor_reduce(
                Ag[:], lnG[:].rearrange("p (s b) -> p b s", b=Bl),
                axis=AX.X, op=OP.add)
            logd = cw.tile([1, Bl], F32, tag="logd")
            nc.scalar.activation(logd[:], dsum[:], ACTF.Ln)

            lr = cw.tile([1, Bl], F32, tag="lr")
            nc.vector.tensor_add(lr[:], logd[:], Aa[:])
            nc.vector.tensor_add(lr[:], lr[:], Ag[:])
            nc.vector.tensor_tensor(lr[:], lr[:], gsum[:], op=OP.subtract)
            nc.sync.dma_start(out=outs["loss"][:].unsqueeze(0), in_=lr[:])
            if cfg.get("DBG"):
                nc.sync.dma_start(out=outs["dqa"][:], in_=qa[:])
                dpg = cw.tile([NT, Bl], F32, tag="dpg")
                nc.vector.tensor_copy(dpg[:], Pg[:])
                nc.sync.dma_start(out=outs["dpg"][:], in_=dpg[:])
                nc.sync.dma_start(out=outs["dha"][:], in_=hista[:])
                nc.sync.dma_start(out=outs["dhg"][:], in_=histg[:])
                dds = cw.tile([1, Bl], F32, tag="dds")
                nc.vector.tensor_copy(dds[:], dsum[:])
                nc.sync.dma_start(out=outs["ddsum"][:], in_=dds[:])
                dgs = cw.tile([1, Bl], F32, tag="dgs")
                nc.vector.tensor_copy(dgs[:], gsum[:])
                nc.sync.dma_start(out=outs["dgsum"][:], in_=dgs[:])
                nc.sync.dma_start(out=outs["dlogd"][:], in_=logd[:])


# ======================= host-side preparation =======================

def make_core_inputs(cfg, x, tags, mask, emb, Wih_f, Whh_f, bih_f, bhh_f,
                     Wih_b, Whh_b, bih_b, bhh_b, W_out, b_out,
                     transitions, start_trans, end_trans):
    """Per-core input map (numpy). x/tags/mask are the LOCAL [Bl, T] slices."""
    T, Bl, NT, Hd = cfg["T"], cfg["Bl"], cfg["NT"], cfg["Hd"]
    R = T * Bl
    M = R // 128
    perm = [0, 1, 3, 2]  # torch gate order (i,f,g,o) -> ours (i,f,o,g)

    WIH_S = np.array([1.0, 1.0, 1.0, 2.0], np.float32)   # (i,f,o,g)
    WHH_S = np.array([2.0, 2.0, 2.0, 4.0], np.float32)

    def reorder_rows(w, scales):
        blocks = [w[k * Hd:(k + 1) * Hd] * s for k, s in zip(perm, scales)]
        return np.concatenate(blocks, axis=0)

    def pack_w(wf, wb, scales):
        out = np.empty((128, 2, 4 * Hd), dtype=ml_dtypes.bfloat16)
        out[:, 0, :] = reorder_rows(np.asarray(wf, np.float32), scales).T
        out[:, 1, :] = reorder_rows(np.asarray(wb, np.float32), scales).T
        return out

    def pack_bias(bi, bh):  # -> [4, 128] rows = gates (i,f,o,g)
        b = np.asarray(bi, np.float32) + np.asarray(bh, np.float32)
        return reorder_rows(b, WIH_S).reshape(4, Hd)

    bias8 = np.empty((8, 128), np.float32)
    bias8[0:4] = pack_bias(bih_f, bhh_f)
    bias8[4:8] = pack_bias(bih_b, bhh_b)
    bias8 = bias8.reshape(1, 8 * 128)

    W_out = np.asarray(W_out, np.float32) * 2.0   # h stored as h/2
    wout = np.empty((128, 2, NT), dtype=ml_dtypes.bfloat16)
    wout[:, 0, :] = W_out[:, :Hd].T
    wout[:, 1, :] = W_out[:, Hd:].T

    x = np.asarray(x)
    tags = np.asarray(tags)
    maskf = np.asarray(mask).astype(np.float32)
    trans = np.ascontiguousarray(transitions, np.float32)
    transb = trans.astype(ml_dtypes.bfloat16)
    start_trans = np.asarray(start_trans, np.float32)
    end_trans = np.asarray(end_trans, np.float32)

    x_tm = x.T.reshape(-1).astype(np.int32)
    idx = x_tm.reshape(M, 128).T.copy()

    eye = np.eye(NT, dtype=np.float32)
    w = maskf.copy()
    w[:, 0] = 1.0
    w1 = eye[tags]
    w1 = (w1 * w[:, :, None]).transpose(2, 1, 0)
    w1hot = np.ascontiguousarray(w1.reshape(NT, R), np.float32)

    sel1 = eye[tags[:, :-1]].transpose(2, 1, 0).reshape(NT, (T - 1) * Bl)
    sel1 = np.ascontiguousarray(sel1, np.float32)
    sel2 = eye[tags[:, 1:]] * maskf[:, 1:, None]
    sel2m = np.ascontiguousarray(
        sel2.transpose(2, 1, 0).reshape(NT, (T - 1) * Bl), np.float32)

    selstart = np.ascontiguousarray(eye[tags[:, 0]].T, np.float32)
    lens = np.asarray(mask).sum(axis=1).astype(np.int64)
    last_tags = tags[np.arange(Bl), lens - 1]
    selend = np.ascontiguousarray(eye[last_tags].T, np.float32)

    IEV = cfg["IEV"]
    injsel = np.zeros((1, (len(IEV) + 1) * Bl), np.float32)
    injsel[0, 0:Bl] = (lens == T).astype(np.float32)
    for e, te in enumerate(IEV):
        injsel[0, (e + 1) * Bl:(e + 2) * Bl] = \
            (lens == te).astype(np.float32)

    return {
        "emb": np.ascontiguousarray(emb, np.float32),
        "idx": idx,
        "wih": pack_w(Wih_f, Wih_b, WIH_S),
        "whh": pack_w(Whh_f, Whh_b, WHH_S),
        "wout": wout,
        "bias8": bias8,
        "bout": np.asarray(b_out, np.float32).reshape(NT, 1),
        "trans": transb,
        "Etrans": np.exp(trans).astype(np.float32),
        "EtransT": np.ascontiguousarray(np.exp(trans).T, np.float32),
        "estart": np.exp(start_trans).astype(np.float32).reshape(NT, 1),
        "einj": np.exp(end_trans).astype(np.float32).reshape(1, NT),
        "injsel": injsel,
        "selstart": selstart,
        "selend": selend,
        "startv": start_trans.reshape(NT, 1).astype(np.float32),
        "endv": end_trans.reshape(NT, 1).astype(np.float32),
        "w1hot": w1hot.astype(ml_dtypes.bfloat16),
        "sel1": sel1.astype(ml_dtypes.bfloat16),
        "sel2m": sel2m.astype(ml_dtypes.bfloat16),
    }


def input_specs(cfg):
    T, Bl, NT, Hd, V = cfg["T"], cfg["Bl"], cfg["NT"], cfg["Hd"], cfg["V"]
    R = T * Bl
    M = R // 128
    NE = len(cfg["IEV"])
    return {
        "emb": ([V, 128], F32),
        "idx": ([128, M], I32),
        "wih": ([128, 2, 4 * Hd], BF16),
        "whh": ([128, 2, 4 * Hd], BF16),
        "wout": ([128, 2, NT], BF16),
        "bias8": ([1, 8 * 128], F32),
        "bout": ([NT, 1], F32),
        "trans": ([NT, NT], BF16),
        "Etrans": ([NT, NT], F32),
        "EtransT": ([NT, NT], F32),
        "estart": ([NT, 1], F32),
        "einj": ([1, NT], F32),
        "injsel": ([1, (NE + 1) * Bl], F32),
        "selstart": ([NT, Bl], F32),
        "selend": ([NT, Bl], F32),
        "startv": ([NT, 1], F32),
        "endv": ([NT, 1], F32),
        "w1hot": ([NT, R], BF16),
        "sel1": ([NT, (T - 1) * Bl], BF16),
        "sel2m": ([NT, (T - 1) * Bl], BF16),
    }


_BUILT = {}


def build_program(cfg, num_devices=8):
    key = tuple((k, v) for k, v in sorted(cfg.items()))
    if key in _BUILT:
        return _BUILT[key]
    nc = bacc.Bacc("TRN2", target_bir_lowering=False, debug=False,
                   num_devices=num_devices)
    ins = {}
    for name, (shape, dt_) in input_specs(cfg).items():
        ins[name] = nc.dram_tensor(name, shape, dt_, kind="ExternalInput").ap()
    outs = {"loss": nc.dram_tensor("loss", [cfg["Bl"]], F32,
                                   kind="ExternalOutput").ap()}
    if cfg.get("DBG"):
        NT_, Bl_, NSH_ = 24, cfg["Bl"], 64
        for nm, shp in [("dqa", [NT_, Bl_]), ("dpg", [NT_, Bl_]),
                        ("dha", [1, NSH_ * Bl_]), ("dhg", [1, NSH_ * Bl_]),
                        ("ddsum", [1, Bl_]), ("dgsum", [1, Bl_]),
                        ("dlogd", [1, Bl_])]:
            outs[nm] = nc.dram_tensor(nm, shp, F32,
                                      kind="ExternalOutput").ap()
    with tile.TileContext(nc) as tc:
        build_body(tc, outs, ins, cfg)
    nc.compile()
    _BUILT[key] = nc
    return nc


def kernel(**inputs):
    from concourse.bass_utils import run_bass_kernel_spmd

    cfg = full_cfg()
    Bl = cfg["Bl"]
    B = 128
    T = cfg["T"]
    n_cores = B // Bl

    np_in = {k: np.asarray(v) for k, v in inputs.items()}
    lens = np_in["mask"].sum(axis=1).astype(np.int64)
    iev = tuple(sorted({int(l) for l in lens if l < T}))
    cfg = dict(cfg, IEV=iev)
    nc = build_program(cfg, num_devices=n_cores)
    in_maps = []
    for c in range(n_cores):
        sl = slice(c * Bl, (c + 1) * Bl)
        in_maps.append(make_core_inputs(
            cfg,
            np_in["x"][sl], np_in["tags"][sl], np_in["mask"][sl],
            np_in["emb"],
            np_in["Wih_f"], np_in["Whh_f"], np_in["bih_f"], np_in["bhh_f"],
            np_in["Wih_b"], np_in["Whh_b"], np_in["bih_b"], np_in["bhh_b"],
            np_in["W_out"], np_in["b_out"], np_in["transitions"],
            np_in["start_trans"], np_in["end_trans"]))

    res = run_bass_kernel_spmd(nc, in_maps, core_ids=list(range(n_cores)),
                               trace=TRACE)
    if res.exec_time_ns is not None:
        LAST_EXEC_NS.append(res.exec_time_ns)
    vals = np.concatenate([res.results[c]["loss"] for c in range(n_cores)])
    return np.float32(vals.mean())


TRACE = False
LAST_EXEC_NS = []
